# revision 38
# baseline (speedup 1.0000x reference)
"""Causal self-attention (b=4, s=2048, d=1024, h=16, hd=64) on 8 trn2 cores.

Sharding: (batch, head-group) — core c handles batch c//2 and heads
[8*(c%2), 8*(c%2)+8) (Megatron column-parallel QKV + row-parallel O).
Each core returns a partial (2048, 1024) output for its batch; the host
sums the two partials per batch (the row-parallel reduce of the Megatron
pattern, done as part of unsharding).

All matmul operands are bf16 (full PE rate like fp32r, but half the
DMA/SBUF traffic and no N>=256 full-rate constraint); accumulation is
fp32 in PSUM, biases fp32.  The output is written bf16 and upcast on
the host.

Per-core device program (layouts chosen so NO on-chip transposes are
needed):
    xT (1024,2048) = x[b].T feeds both Q^T/K^T (as moving operand) and
    V (as stationary operand).  Q^T/K^T stored [o=512 part-dims, n];
    V stored [n part, o free] with a ones column per head so the softmax
    denominator falls out of the PV matmul (M=65).  probs kept
    TRANSPOSED [kv, q]: softmax needs no max-subtraction (scores bounded
    ~|3|), the causal mask is additive (-1e4 pre-exp, exp underflows to
    0), and attn^T [u, n] is directly the stationary operand of the
    O-projection.  Causality: fully-masked kv-chunks are skipped
    entirely, and on diagonal chunks the fully-masked column range is
    never computed.

    Schedule: 5 phases; phase p emits the projections of x-slabs
    (2p, 2p+1) INTERLEAVED with the attention of q-chunk p-1 and its
    O-projection.  Scores for a head pair land in ONE merged [128,2,512]
    PSUM tile (2 banks) so the mask-add and exp are single instructions
    over both heads.  PSUM: 2x merged score tiles (4 banks, shared with
    the projection accumulators) + 4 PV accumulator banks.
"""
from contextlib import ExitStack

import numpy as np

MM_MODE = "bf16"  # kept for test.py compat; only bf16 path exists


def _build(repeat=1, ratio=(2, 1), lag=3):
    import concourse.tile as tile
    from concourse import bacc, mybir

    dt = mybir.dt
    F32 = dt.float32
    B16 = dt.bfloat16
    Exp = mybir.ActivationFunctionType.Exp
    Copy = mybir.ActivationFunctionType.Copy
    Identity = mybir.ActivationFunctionType.Identity

    nc = bacc.Bacc("TRN2", target_bir_lowering=False, debug=False, num_devices=8)

    xT = nc.dram_tensor("xT", [8, 128, 8, 256], B16, kind="ExternalInput").ap()
    wqkT = nc.dram_tensor("wqkT", [128, 8, 1024], B16, kind="ExternalInput").ap()
    wvT = nc.dram_tensor("wvT", [128, 8, 512], B16, kind="ExternalInput").ap()
    woT = nc.dram_tensor("woT", [128, 4, 1024], B16, kind="ExternalInput").ap()
    bqk = nc.dram_tensor("bqk", [128, 16], F32, kind="ExternalInput").ap()
    bvb = nc.dram_tensor("bvb", [128, 512], F32, kind="ExternalInput").ap()
    bob = nc.dram_tensor("bob", [128, 1024], F32, kind="ExternalInput").ap()
    maskt = nc.dram_tensor("maskt", [128, 256], F32, kind="ExternalInput").ap()
    out = nc.dram_tensor("out", [2048, 1024], B16, kind="ExternalOutput").ap()

    outr = out.rearrange("(nc p) o -> p nc o", p=128)    # [128, 16, 1024]

    with tile.TileContext(nc) as tc, ExitStack() as ctx:
        big = ctx.enter_context(tc.tile_pool(name="big", bufs=1))
        pqt = ctx.enter_context(tc.tile_pool(name="pqt", bufs=1))
        pkt = ctx.enter_context(tc.tile_pool(name="pkt", bufs=1))
        pv = ctx.enter_context(tc.tile_pool(name="pv", bufs=1))
        pxs = ctx.enter_context(tc.tile_pool(name="pxs", bufs=3))
        pprob = ctx.enter_context(tc.tile_pool(name="pprob", bufs=4))
        precb = ctx.enter_context(tc.tile_pool(name="precb", bufs=1))
        prd = ctx.enter_context(tc.tile_pool(name="prd", bufs=1))
        prd4 = ctx.enter_context(tc.tile_pool(name="prd4", bufs=2))
        pone = ctx.enter_context(tc.tile_pool(name="pone", bufs=1))
        pout = ctx.enter_context(tc.tile_pool(name="pout", bufs=2))
        poba = ctx.enter_context(tc.tile_pool(name="poba", bufs=8))
        paun = ctx.enter_context(tc.tile_pool(name="paun", bufs=4))
        patq = ctx.enter_context(tc.tile_pool(name="patq", bufs=2))
        psmm = ctx.enter_context(tc.tile_pool(name="psmm", bufs=2, space="PSUM"))
        pprj = ctx.enter_context(tc.tile_pool(name="pprj", bufs=2, space="PSUM"))
        pspv = ctx.enter_context(tc.tile_pool(name="pspv", bufs=2, space="PSUM"))

        # ---- constants (one merged tile: bqk | ones8 | bvb | bob | mask2) ----
        const_sb = pone.tile([128, 1808], F32, tag="const")
        bqk_sb = const_sb[:, 0:8]
        ones8_sb = const_sb[:, 8:16]
        bvb_sb = const_sb[:, 16:528]
        bob_sb = const_sb[:, 528:1552]
        tri2_sb = const_sb[:, 1552:1808].rearrange("p (two e) -> p two e", e=128)

        for rep in range(repeat):
            # prefetch the first two x slabs so projections start ASAP
            xs0 = pxs.tile([128, 8, 256], B16, tag="xs")
            nc.sync.dma_start(out=xs0[:, 0:4], in_=xT[0, :, 0:4])
            # ---- weights ----
            wv_sb = big.tile([128, 8, 512], B16, tag="bigB")
            nc.sync.dma_start(out=wv_sb[:, 0:4], in_=wvT[:, 0:4])
            nc.sync.dma_start(out=xs0[:, 4:8], in_=xT[0, :, 4:8])
            nc.sync.dma_start(out=wv_sb[:, 4:8], in_=wvT[:, 4:8])
            if rep == 0:
                nc.sync.dma_start(out=const_sb[:, 0:16], in_=bqk)
                nc.sync.dma_start(out=bvb_sb, in_=bvb)
            xs1 = pxs.tile([128, 8, 256], B16, tag="xs", name="xs1")
            nc.sync.dma_start(out=xs1[:, 0:4], in_=xT[1, :, 0:4])
            nc.sync.dma_start(out=xs1[:, 4:8], in_=xT[1, :, 4:8])
            wqk_sb = big.tile([128, 8, 1024], B16, tag="bigA")
            for kc in range(8):
                nc.sync.dma_start(out=wqk_sb[:, kc], in_=wqkT[:, kc])
            if rep == 0:
                nc.sync.dma_start(out=bob_sb, in_=bob)
                nc.sync.dma_start(out=const_sb[:, 1552:1808], in_=maskt)
            wo_sb = big.tile([128, 4, 1024], B16, tag="bigC")
            nc.sync.dma_start(out=wo_sb[:], in_=woT)

            # ---- persistent activations ----
            qt = pqt.tile([128, 4, 2048], B16)   # Q^T: u-dim on partitions
            kt = pkt.tile([128, 4, 2048], B16)   # K^T
            vt = pv.tile([128, 16, 520], B16)    # V: [n part, 8*(64+ones)]

            # 5 phases: phase p emits projections for slabs (2p, 2p+1)
            # INTERLEAVED with the attention of q-chunk p-1; the O-proj of
            # chunk p-2 rides along one phase later (its normalize is long
            # done by then, so it never stalls the PE).
            def proj_units(sp):
                units = []

                def mk_dma(ns):
                    def dma_u():
                        xs = pxs.tile([128, 8, 256], B16, tag="xs", name=f"xs{ns}")
                        nc.sync.dma_start(out=xs[:, 0:4], in_=xT[ns, :, 0:4])
                        nc.sync.dma_start(out=xs[:, 4:8], in_=xT[ns, :, 4:8])
                        xss[ns] = xs
                    return dma_u

                def mk_v(ns, nn):
                    def v_u():
                        ni = 2 * ns + nn
                        pmv = pprj.tile([128, 512], F32, tag="mm", name="pmv")
                        for kc in range(8):
                            nc.tensor.matmul(
                                pmv[:],
                                xss[ns][:, kc, 128 * nn:128 * (nn + 1)],
                                wv_sb[:, kc, :],
                                start=(kc == 0), stop=(kc == 7),
                            )
                        vslab = vt[:, ni, :].rearrange("p (h e) -> p h e", e=65)
                        nc.vector.tensor_copy(out=vslab[:, :, 64], in_=ones8_sb)
                        nc.vector.tensor_add(
                            vslab[:, :, 0:64],
                            pmv[:].rearrange("p (h e) -> p h e", e=64),
                            bvb_sb.rearrange("p (h e) -> p h e", e=64),
                        )
                    return v_u

                def mk_qk(ns, oc):
                    def qk_u():
                        pm = pprj.tile([128, 256], F32, tag="mm", name="pmqk")
                        for kc in range(8):
                            nc.tensor.matmul(
                                pm[:],
                                wqk_sb[:, kc, 128 * oc:128 * (oc + 1)],
                                xss[ns][:, kc, :],
                                start=(kc == 0), stop=(kc == 7),
                            )
                        dest = qt if oc < 4 else kt
                        nc.vector.tensor_scalar_add(
                            dest[:, oc % 4, 256 * ns:256 * (ns + 1)], pm[:],
                            bqk_sb[:, oc:oc + 1],
                        )
                    return qk_u

                # x slabs for THIS phase were prefetched last phase; here we
                # prefetch the next phase's two slabs.
                if sp == 0:
                    # V first: needs only xs+wv (the first DMAs to land);
                    # the QK units then overlap the wqk stream.
                    units.extend([mk_v(0, 0), mk_v(0, 1), mk_v(1, 0), mk_v(1, 1)])
                    units.append(mk_dma(2))
                    units.extend([mk_qk(0, oc) for oc in range(8)])
                    units.append(mk_dma(3))
                    units.extend([mk_qk(1, oc) for oc in range(8)])
                else:
                    if sp < 3:
                        units.append(mk_dma(2 * sp + 2))
                    for ns in (2 * sp, 2 * sp + 1):
                        units.extend([mk_qk(ns, oc) for oc in range(8)])
                        units.extend([mk_v(ns, 0), mk_v(ns, 1)])
                        if sp < 3 and ns == 2 * sp:
                            units.append(mk_dma(2 * sp + 3))
                return units

            def attn_units(sp, atq, hps=(0, 1, 2, 3), piecewise_norm=(),
                           batched=False, nb_sink=None):
                q0 = 512 * sp
                J = 4 * (sp + 1)
                LAG = lag   # PV of step j is emitted inside step j+LAG's unit
                units = []
                # Sequential head-pair chains; each chain's PV runs LAG steps
                # behind its S_T/exp so the PE never waits on the exp.
                batch = {"rd4": None, "rows": []}

                def mk_norm_batch(batch=batch):
                    # One DVE reciprocal for 4 collected denominators (at
                    # partition bases 0/32/64/96), then broadcast + in-place
                    # multiply of the already-copied bf16 numerators in atq.
                    def nb_u():
                        rd4 = batch["rd4"]
                        rr4 = prd.tile([128, 512], F32, tag="rr4", name="rr4")
                        nc.vector.reciprocal(rr4[:], rd4[:])
                        for (hp, half, r, aun) in batch["rows"]:
                            po = 64 * half
                            # partition_broadcast only works from/to base 0
                            # on HW: stage the row at base 0 via ACT first.
                            st0 = prd.tile([1, 512], F32, tag="st", name="st")
                            nc.scalar.activation(out=st0[:], in_=rr4[r:r + 1, :],
                                                 func=Copy)
                            rb = precb.tile([128, 512], F32, tag="rb", name="rb")
                            nc.gpsimd.partition_broadcast(rb[0:64, :], st0[:])
                            nc.vector.tensor_mul(
                                atq[po:po + 64, hp, :],
                                aun[0:64, :], rb[0:64, :])
                        batch["rd4"] = None
                        batch["rows"] = []
                    return nb_u

                for hp in hps:
                    st = {"pvps": None, "pend": []}

                    def norm_piece(hp, st, lo, hi):
                        for half in range(2):
                            po = 64 * half
                            pvp = st["pvps"][half]
                            rd = prd.tile([1, 512], F32, tag="rd", name="rd")
                            nc.vector.reciprocal(rd[:, lo:hi],
                                                 pvp[64:65, lo:hi])
                            rb = precb.tile([128, 512], F32, tag="rb", name="rb")
                            nc.gpsimd.partition_broadcast(rb[0:64, lo:hi],
                                                          rd[:, lo:hi])
                            nc.vector.tensor_mul(
                                atq[po:po + 64, hp, lo:hi],
                                pvp[0:64, lo:hi], rb[0:64, lo:hi])

                    def norm_defer(hp, st, batch=batch):
                        # Chain end for a batched chunk: ACT-copies the bf16
                        # numerator into atq and the denominator into the
                        # shared rd4 collection tile; division happens in the
                        # next norm-batch unit (off the critical path).
                        if batch["rd4"] is None:
                            batch["rd4"] = prd4.tile([128, 512], F32,
                                                     tag="rd4", name="rd4")
                            nc.vector.memset(batch["rd4"][:], 1.0)
                        for half in range(2):
                            pvp = st["pvps"][half]
                            aun = paun.tile([64, 512], B16, tag="aun",
                                            name="aun")
                            nc.scalar.activation(
                                out=aun[:], in_=pvp[0:64, :], func=Copy)
                            r = 64 * (hp % 2) + 32 * half
                            nc.scalar.activation(
                                out=batch["rd4"][r:r + 1, :],
                                in_=pvp[64:65, :], func=Copy)
                            batch["rows"].append((hp, half, r, aun))

                    def emit_pv(hp, st, last, piecewise=False):
                        pj, ppt, pc0 = st["pend"].pop(0)
                        if pj == 0:
                            st["pvps"] = [
                                pspv.tile([65, 512], F32, tag="pv", name="pvpa"),
                                pspv.tile([65, 512], F32, tag="pv", name="pvpb"),
                            ]
                        for half in range(2):
                            h = 2 * hp + half
                            nc.tensor.matmul(
                                st["pvps"][half][:, pc0:512],
                                vt[:, pj, 65 * h:65 * h + 65],
                                ppt[:, half, pc0:512],
                                start=(pj == 0), stop=last,
                            )
                        # columns [128*toff, 128*toff+128) got their final PV
                        # contribution: normalize them right away so the
                        # O-projection never waits on a monolithic normalize.
                        if piecewise:
                            toff = pj - 4 * sp
                            if toff >= 0:
                                norm_piece(hp, st, 128 * toff, 128 * toff + 128)

                    def mk_step(hp, j, st=st):
                        def step_u():
                            toff = j - 4 * sp
                            c0 = 128 * toff if toff > 0 else 0
                            sm = psmm.tile([128, 2, 512], F32, tag="sm", name="sm")
                            for half in range(2):  # head 2hp+half in PE band
                                po = 64 * half
                                nc.tensor.matmul(
                                    sm[:, half, c0:512],
                                    kt[po:po + 64, hp, 128 * j:128 * (j + 1)],
                                    qt[po:po + 64, hp, q0 + c0:q0 + 512],
                                    start=True, stop=True,
                                )
                            if toff >= 0:  # diagonal: triangle add (both)
                                nc.vector.tensor_add(
                                    sm[:, :, c0:c0 + 128], sm[:, :, c0:c0 + 128],
                                    tri2_sb)
                            pt = pprob.tile([128, 2, 512], B16, tag="pt", name="pt")
                            nc.scalar.activation(
                                out=pt[:, :, c0:512], in_=sm[:, :, c0:512],
                                func=Exp, scale=0.125)
                            st["pend"].append((j, pt, c0))
                            if len(st["pend"]) > LAG:
                                emit_pv(hp, st, last=False,
                                        piecewise=hp in piecewise_norm)
                        return step_u

                    def mk_flush(hp, st=st):
                        def flush_u():
                            pw = hp in piecewise_norm
                            while st["pend"]:
                                emit_pv(hp, st, last=not st["pend"][1:],
                                        piecewise=pw)
                            if pw:
                                return
                            if batched:
                                norm_defer(hp, st)
                            else:  # normalize both heads in one go
                                norm_piece(hp, st, 0, 512)
                        return flush_u

                    for j in range(J):
                        units.append(mk_step(hp, j))
                    units.append(mk_flush(hp))
                    if batched and hp % 2 == 1:
                        units.append(mk_norm_batch())
                return units

            def o_units(sp, atq):
                units = []
                for k in range(4):
                    for oh in range(2):
                        def o_u(k=k, oh=oh):
                            ni = 4 * sp + k
                            pm = pprj.tile([128, 512], F32, tag="mm", name="pmo")
                            for uc in range(4):
                                nc.tensor.matmul(
                                    pm[:],
                                    atq[:, uc, 128 * k:128 * (k + 1)],
                                    wo_sb[:, uc, 512 * oh:512 * (oh + 1)],
                                    start=(uc == 0), stop=(uc == 3),
                                )
                            ob = pout.tile([128, 512], B16, tag="ob", name="ob")
                            nc.vector.tensor_add(
                                ob[:], pm[:], bob_sb[:, 512 * oh:512 * (oh + 1)])
                            nc.scalar.dma_start(
                                out=outr[:, ni, 512 * oh:512 * (oh + 1)], in_=ob[:])
                        units.append(o_u)
                return units

            def o_units_split(sp, atq):
                """O-proj split in two half-accumulations: the uc 0/1 part
                can run while head-pairs 2/3 are still in attention."""
                obas = {}
                ua, ub = [], []
                for k in range(4):
                    for oh in range(2):
                        def oa_u(k=k, oh=oh):
                            pm = pprj.tile([128, 512], F32, tag="mm", name="pmoa")
                            for uc in range(2):
                                nc.tensor.matmul(
                                    pm[:],
                                    atq[:, uc, 128 * k:128 * (k + 1)],
                                    wo_sb[:, uc, 512 * oh:512 * (oh + 1)],
                                    start=(uc == 0), stop=(uc == 1),
                                )
                            oba = poba.tile([128, 512], B16, tag="oba", name="oba")
                            nc.vector.tensor_add(
                                oba[:], pm[:], bob_sb[:, 512 * oh:512 * (oh + 1)])
                            obas[(k, oh)] = oba

                        def ob_u(k=k, oh=oh):
                            ni = 4 * sp + k
                            pm = pprj.tile([128, 512], F32, tag="mm", name="pmob")
                            for uc in range(2, 4):
                                nc.tensor.matmul(
                                    pm[:],
                                    atq[:, uc, 128 * k:128 * (k + 1)],
                                    wo_sb[:, uc, 512 * oh:512 * (oh + 1)],
                                    start=(uc == 2), stop=(uc == 3),
                                )
                            ob = pout.tile([128, 512], B16, tag="ob", name="ob")
                            nc.vector.tensor_add(ob[:], pm[:], obas[(k, oh)][:])
                            nc.scalar.dma_start(
                                out=outr[:, ni, 512 * oh:512 * (oh + 1)], in_=ob[:])
                        ua.append(oa_u)
                        ub.append(ob_u)
                return ua, ub

            def run_interleaved(cur, prev):
                # proportional round-robin interleave of cur and prev
                na, nb = len(cur), len(prev)
                ia = ib = 0
                while ia < na or ib < nb:
                    if ib * max(na, 1) * ratio[1] <= ia * max(nb, 1) * ratio[0] and ib < nb or ia >= na:
                        prev[ib](); ib += 1
                    else:
                        cur[ia](); ia += 1

            xss = {0: xs0, 1: xs1}
            atqs = {}
            for sp in range(1, 4):
                atqs[sp - 1] = None
            nb_pend = []
            for sp in range(4):
                cur = list(nb_pend)
                nb_pend = []
                cur += proj_units(sp)
                if sp >= 2:
                    cur = cur + o_units(sp - 2, atqs[sp - 2])
                prev = []
                if sp >= 1:
                    atqs[sp - 1] = patq.tile([128, 4, 512], B16, tag="atq",
                                             name=f"atq{sp - 1}")
                    prev = attn_units(sp - 1, atqs[sp - 1], batched=True)
                run_interleaved(cur, prev)
            # phase 4: attention chunk 3; O(2) rides the first two chains,
            # O(3)'s uc0/1 half rides the last two, its uc2/3 half drains.
            atqs[3] = patq.tile([128, 4, 512], B16, tag="atq", name="atq3")
            o3a, o3b = o_units_split(3, atqs[3])
            run_interleaved(nb_pend + o_units(2, atqs[2]),
                            attn_units(3, atqs[3], hps=(0, 1), batched=True))
            run_interleaved(o3a,
                            attn_units(3, atqs[3], hps=(2, 3), batched=True))
            for u in o3b:
                u()

    nc.compile()
    return nc


_NC_CACHE = {}


def _get_nc(repeat=1, **kw):
    key = (repeat, tuple(sorted(kw.items())))
    if key not in _NC_CACHE:
        _NC_CACHE[key] = _build(repeat, **kw)
    return _NC_CACHE[key]


def _host_inputs(x, Wq, bq, Wk, bk, Wv, bv, Wo, bo):
    """Build the 8 per-core input maps."""
    import ml_dtypes
    f32 = np.float32
    B16 = ml_dtypes.bfloat16

    def rnd(a):
        return np.ascontiguousarray(a, dtype=f32).astype(B16)

    r = np.arange(128)[:, None]
    c = np.arange(128)[None, :]
    mask1 = np.where(r <= c, f32(0.0), f32(-1e4)).astype(f32)
    mask = np.concatenate([mask1, mask1], axis=1)

    in_maps = []
    for core in range(8):
        bi, hg = core // 2, core % 2
        hsl = slice(512 * hg, 512 * (hg + 1))
        # xT swizzled: [ns, p, kc, col] = x[bi].T[kc*128+p, 256*ns+col]
        xTl = rnd(np.ascontiguousarray(
            x[bi].T.reshape(8, 128, 8, 256).transpose(2, 1, 0, 3)))
        wqkTl = rnd(np.ascontiguousarray(
            np.concatenate([Wq[hsl].T, Wk[hsl].T], axis=1).reshape(8, 128, 1024)
            .transpose(1, 0, 2)))
        wvTl = rnd(np.ascontiguousarray(
            Wv[hsl].T.reshape(8, 128, 512).transpose(1, 0, 2)))
        woTl = rnd(np.ascontiguousarray(
            Wo[:, hsl].T.reshape(4, 128, 1024).transpose(1, 0, 2)))
        bq_l, bk_l = bq[hsl], bk[hsl]
        bqk_t = np.stack(
            [bq_l[128 * i:128 * (i + 1)] for i in range(4)]
            + [bk_l[128 * i:128 * (i + 1)] for i in range(4)]
            + [np.ones(128, dtype=f32)] * 8, axis=1
        ).astype(f32)
        bvb_t = np.broadcast_to(bv[hsl].astype(f32), (128, 512)).copy()
        if hg == 0:
            bob_t = np.broadcast_to(bo.astype(f32), (128, 1024)).copy()
        else:
            bob_t = np.zeros((128, 1024), dtype=f32)
        in_maps.append({
            "xT": xTl, "wqkT": wqkTl, "wvT": wvTl, "woT": woTl,
            "bqk": bqk_t, "bvb": bvb_t, "bob": bob_t, "maskt": mask,
        })
    return in_maps


def kernel(x, Wq, bq, Wk, bk, Wv, bv, Wo, bo):
    from concourse.bass_utils import run_bass_kernel_spmd

    x = np.asarray(x); Wq = np.asarray(Wq); bq = np.asarray(bq)
    Wk = np.asarray(Wk); bk = np.asarray(bk); Wv = np.asarray(Wv)
    bv = np.asarray(bv); Wo = np.asarray(Wo); bo = np.asarray(bo)

    nc = _get_nc()
    in_maps = _host_inputs(x, Wq, bq, Wk, bk, Wv, bv, Wo, bo)
    r = run_bass_kernel_spmd(nc, in_maps, list(range(8)))

    out = np.empty((4, 2048, 1024), dtype=np.float32)
    for bi in range(4):
        out[bi] = (r.results[2 * bi]["out"].astype(np.float32)
                   + r.results[2 * bi + 1]["out"].astype(np.float32))
    return out


# revision 40
# speedup vs baseline: 1.0485x; 1.0485x over previous
"""Causal self-attention (b=4, s=2048, d=1024, h=16, hd=64) on 8 trn2 cores.

Sharding: (batch, head-group) — core c handles batch c//2 and heads
[8*(c%2), 8*(c%2)+8) (Megatron column-parallel QKV + row-parallel O).
Each core returns a partial (2048, 1024) bf16 output for its batch; the
host upcasts and sums the two partials per batch (the row-parallel
reduce of the Megatron pattern, done as part of unsharding).

All matmul operands are bf16 (full PE rate like fp32r, half the
DMA/SBUF traffic, and no N>=256 full-rate constraint); accumulation is
fp32 in PSUM, biases fp32.

Per-core device program (layouts chosen so NO on-chip transposes are
needed):
    xT (1024,2048) = x[b].T feeds both Q^T/K^T (as moving operand) and
    V (as stationary operand).  Q^T/K^T stored [o=512 part-dims, n];
    V stored [n part, o free] with a ones column per head so the softmax
    denominator falls out of the PV matmul (M=65).  probs kept
    TRANSPOSED [kv, q]: softmax needs no max-subtraction (scores bounded
    ~|3|), the causal mask is additive (-1e4 pre-exp, exp underflows to
    0), and attn^T [u, n] is directly the stationary operand of the
    O-projection.  Causality: fully-masked kv-chunks are skipped
    entirely; on diagonal chunks only the live column range is computed.

    Schedule: 5 phases; phase p emits the projections of x-slabs
    (2p, 2p+1) interleaved with the attention of q-chunk p-1; the
    O-projection of chunk p-2 rides along one phase later so it never
    waits on the softmax normalize.  Attention runs as sequential
    head-pair chains whose PV matmuls lag the S_T/exp stream by `lag`
    steps, so the in-order PE never waits on the ACT exp.  Scores for a
    head pair land in ONE merged [128,2,512] PSUM tile, so mask-add and
    exp are single instructions over both heads.  PSUM: 2 merged score
    tiles (4 banks) + 2 projection accumulators (2 banks) + 2 PV
    accumulator banks.

    Softmax normalize: nc.vector.reciprocal is ~2.8us on HW (multi-pass)
    and blocks the in-order DVE queue, so denominators are collected
    across two chains into one [128,512] tile (rows 0/32/64/96 — the
    only partition bases single-row ACT/DVE ops support) via cheap ACT
    copies, reciprocal'd in ONE DVE op, then broadcast+multiplied into
    the (ACT-copied) bf16 numerators off the critical path.  The last
    chunk's final chain instead normalizes piecewise per 128-column
    group as its diagonal PVs complete, so the O-projection tail starts
    immediately.  (gpsimd partition_broadcast only works from/to
    partition base 0 on HW; ACT stages rows there first.)

    The final chunk's O-projection is split into uc{0,1} and uc{2,3}
    half-accumulations so its first half overlaps the last two
    attention chains.

    Measured (NTFF, single dispatch, core 0): ~331 us at throttle-util
    0.85 (~282 us unthrottled-equivalent) vs 577 us (at util 0.60;
    ~349 us equivalent) for the fp32r predecessor.  Cost-model
    prediction (no throttle): ~270 us.  rel err vs fp32 reference:
    4.1e-3 (threshold 2e-2).
"""
from contextlib import ExitStack

import numpy as np

MM_MODE = "bf16"  # kept for test.py compat; only bf16 path exists


def _build(repeat=1, ratio=(2, 1), lag=3):
    import concourse.tile as tile
    from concourse import bacc, mybir

    dt = mybir.dt
    F32 = dt.float32
    B16 = dt.bfloat16
    Exp = mybir.ActivationFunctionType.Exp
    Copy = mybir.ActivationFunctionType.Copy
    Identity = mybir.ActivationFunctionType.Identity

    nc = bacc.Bacc("TRN2", target_bir_lowering=False, debug=False, num_devices=8)

    xT = nc.dram_tensor("xT", [8, 128, 8, 256], B16, kind="ExternalInput").ap()
    wqkT = nc.dram_tensor("wqkT", [128, 8, 1024], B16, kind="ExternalInput").ap()
    wvT = nc.dram_tensor("wvT", [128, 8, 512], B16, kind="ExternalInput").ap()
    woT = nc.dram_tensor("woT", [128, 4, 1024], B16, kind="ExternalInput").ap()
    bqk = nc.dram_tensor("bqk", [128, 16], F32, kind="ExternalInput").ap()
    bvb = nc.dram_tensor("bvb", [128, 512], F32, kind="ExternalInput").ap()
    bob = nc.dram_tensor("bob", [128, 1024], F32, kind="ExternalInput").ap()
    maskt = nc.dram_tensor("maskt", [128, 256], F32, kind="ExternalInput").ap()
    out = nc.dram_tensor("out", [2048, 1024], B16, kind="ExternalOutput").ap()

    outr = out.rearrange("(nc p) o -> p nc o", p=128)    # [128, 16, 1024]

    with tile.TileContext(nc) as tc, ExitStack() as ctx:
        big = ctx.enter_context(tc.tile_pool(name="big", bufs=1))
        pqt = ctx.enter_context(tc.tile_pool(name="pqt", bufs=1))
        pkt = ctx.enter_context(tc.tile_pool(name="pkt", bufs=1))
        pv = ctx.enter_context(tc.tile_pool(name="pv", bufs=1))
        pxs = ctx.enter_context(tc.tile_pool(name="pxs", bufs=3))
        pprob = ctx.enter_context(tc.tile_pool(name="pprob", bufs=4))
        precb = ctx.enter_context(tc.tile_pool(name="precb", bufs=1))
        prd = ctx.enter_context(tc.tile_pool(name="prd", bufs=1))
        prd4 = ctx.enter_context(tc.tile_pool(name="prd4", bufs=2))
        pone = ctx.enter_context(tc.tile_pool(name="pone", bufs=1))
        pout = ctx.enter_context(tc.tile_pool(name="pout", bufs=2))
        poba = ctx.enter_context(tc.tile_pool(name="poba", bufs=8))
        paun = ctx.enter_context(tc.tile_pool(name="paun", bufs=4))
        patq = ctx.enter_context(tc.tile_pool(name="patq", bufs=2))
        psmm = ctx.enter_context(tc.tile_pool(name="psmm", bufs=2, space="PSUM"))
        pprj = ctx.enter_context(tc.tile_pool(name="pprj", bufs=2, space="PSUM"))
        pspv = ctx.enter_context(tc.tile_pool(name="pspv", bufs=2, space="PSUM"))

        # ---- constants (one merged tile: bqk | ones8 | bvb | bob | mask2) ----
        const_sb = pone.tile([128, 1808], F32, tag="const")
        bqk_sb = const_sb[:, 0:8]
        ones8_sb = const_sb[:, 8:16]
        bvb_sb = const_sb[:, 16:528]
        bob_sb = const_sb[:, 528:1552]
        tri2_sb = const_sb[:, 1552:1808].rearrange("p (two e) -> p two e", e=128)

        for rep in range(repeat):
            # prefetch the first two x slabs so projections start ASAP
            xs0 = pxs.tile([128, 8, 256], B16, tag="xs")
            nc.sync.dma_start(out=xs0[:, 0:4], in_=xT[0, :, 0:4])
            # ---- weights ----
            wv_sb = big.tile([128, 8, 512], B16, tag="bigB")
            nc.sync.dma_start(out=wv_sb[:, 0:4], in_=wvT[:, 0:4])
            nc.sync.dma_start(out=xs0[:, 4:8], in_=xT[0, :, 4:8])
            nc.sync.dma_start(out=wv_sb[:, 4:8], in_=wvT[:, 4:8])
            if rep == 0:
                nc.sync.dma_start(out=const_sb[:, 0:16], in_=bqk)
                nc.sync.dma_start(out=bvb_sb, in_=bvb)
            xs1 = pxs.tile([128, 8, 256], B16, tag="xs", name="xs1")
            nc.sync.dma_start(out=xs1[:, 0:4], in_=xT[1, :, 0:4])
            nc.sync.dma_start(out=xs1[:, 4:8], in_=xT[1, :, 4:8])
            wqk_sb = big.tile([128, 8, 1024], B16, tag="bigA")
            for kc in range(8):
                nc.sync.dma_start(out=wqk_sb[:, kc], in_=wqkT[:, kc])
            if rep == 0:
                nc.sync.dma_start(out=bob_sb, in_=bob)
                nc.sync.dma_start(out=const_sb[:, 1552:1808], in_=maskt)
            wo_sb = big.tile([128, 4, 1024], B16, tag="bigC")
            nc.sync.dma_start(out=wo_sb[:], in_=woT)

            # ---- persistent activations ----
            qt = pqt.tile([128, 4, 2048], B16)   # Q^T: u-dim on partitions
            kt = pkt.tile([128, 4, 2048], B16)   # K^T
            vt = pv.tile([128, 16, 520], B16)    # V: [n part, 8*(64+ones)]

            # 5 phases: phase p emits projections for slabs (2p, 2p+1)
            # INTERLEAVED with the attention of q-chunk p-1; the O-proj of
            # chunk p-2 rides along one phase later (its normalize is long
            # done by then, so it never stalls the PE).
            def proj_units(sp):
                units = []

                def mk_dma(ns):
                    def dma_u():
                        xs = pxs.tile([128, 8, 256], B16, tag="xs", name=f"xs{ns}")
                        nc.sync.dma_start(out=xs[:, 0:4], in_=xT[ns, :, 0:4])
                        nc.sync.dma_start(out=xs[:, 4:8], in_=xT[ns, :, 4:8])
                        xss[ns] = xs
                    return dma_u

                def mk_v(ns, nn):
                    def v_u():
                        ni = 2 * ns + nn
                        pmv = pprj.tile([128, 512], F32, tag="mm", name="pmv")
                        for kc in range(8):
                            nc.tensor.matmul(
                                pmv[:],
                                xss[ns][:, kc, 128 * nn:128 * (nn + 1)],
                                wv_sb[:, kc, :],
                                start=(kc == 0), stop=(kc == 7),
                            )
                        vslab = vt[:, ni, :].rearrange("p (h e) -> p h e", e=65)
                        nc.vector.tensor_copy(out=vslab[:, :, 64], in_=ones8_sb)
                        nc.vector.tensor_add(
                            vslab[:, :, 0:64],
                            pmv[:].rearrange("p (h e) -> p h e", e=64),
                            bvb_sb.rearrange("p (h e) -> p h e", e=64),
                        )
                    return v_u

                def mk_qk(ns, oc):
                    def qk_u():
                        pm = pprj.tile([128, 256], F32, tag="mm", name="pmqk")
                        for kc in range(8):
                            nc.tensor.matmul(
                                pm[:],
                                wqk_sb[:, kc, 128 * oc:128 * (oc + 1)],
                                xss[ns][:, kc, :],
                                start=(kc == 0), stop=(kc == 7),
                            )
                        dest = qt if oc < 4 else kt
                        nc.vector.tensor_scalar_add(
                            dest[:, oc % 4, 256 * ns:256 * (ns + 1)], pm[:],
                            bqk_sb[:, oc:oc + 1],
                        )
                    return qk_u

                # x slabs for THIS phase were prefetched last phase; here we
                # prefetch the next phase's two slabs.
                if sp == 0:
                    # V first: needs only xs+wv (the first DMAs to land);
                    # the QK units then overlap the wqk stream.
                    units.extend([mk_v(0, 0), mk_v(0, 1), mk_v(1, 0), mk_v(1, 1)])
                    units.append(mk_dma(2))
                    units.extend([mk_qk(0, oc) for oc in range(8)])
                    units.append(mk_dma(3))
                    units.extend([mk_qk(1, oc) for oc in range(8)])
                else:
                    if sp < 3:
                        units.append(mk_dma(2 * sp + 2))
                    units.extend([mk_qk(2 * sp, oc) for oc in range(8)])
                    if sp < 3:
                        units.append(mk_dma(2 * sp + 3))
                    units.extend([mk_qk(2 * sp + 1, oc) for oc in range(8)])
                    units.extend([mk_v(2 * sp, 0), mk_v(2 * sp, 1),
                                  mk_v(2 * sp + 1, 0), mk_v(2 * sp + 1, 1)])
                return units

            def attn_units(sp, atq, hps=(0, 1, 2, 3), piecewise_norm=(),
                           batched=False, nb_sink=None):
                q0 = 512 * sp
                J = 4 * (sp + 1)
                LAG = lag   # PV of step j is emitted inside step j+LAG's unit
                units = []
                # Sequential head-pair chains; each chain's PV runs LAG steps
                # behind its S_T/exp so the PE never waits on the exp.
                batch = {"rd4": None, "rows": []}

                def mk_norm_batch(batch=batch):
                    # One DVE reciprocal for 4 collected denominators (at
                    # partition bases 0/32/64/96), then broadcast + in-place
                    # multiply of the already-copied bf16 numerators in atq.
                    def nb_u():
                        rd4 = batch["rd4"]
                        rr4 = prd.tile([128, 512], F32, tag="rr4", name="rr4")
                        nc.vector.reciprocal(rr4[:], rd4[:])
                        for (hp, half, r, aun) in batch["rows"]:
                            po = 64 * half
                            # partition_broadcast only works from/to base 0
                            # on HW: stage the row at base 0 via ACT first.
                            st0 = prd.tile([1, 512], F32, tag="st", name="st")
                            nc.scalar.activation(out=st0[:], in_=rr4[r:r + 1, :],
                                                 func=Copy)
                            rb = precb.tile([128, 512], F32, tag="rb", name="rb")
                            nc.gpsimd.partition_broadcast(rb[0:64, :], st0[:])
                            nc.vector.tensor_mul(
                                atq[po:po + 64, hp, :],
                                aun[0:64, :], rb[0:64, :])
                        batch["rd4"] = None
                        batch["rows"] = []
                    return nb_u

                for hp in hps:
                    st = {"pvps": None, "pend": []}

                    def norm_piece(hp, st, lo, hi):
                        for half in range(2):
                            po = 64 * half
                            pvp = st["pvps"][half]
                            rd = prd.tile([1, 512], F32, tag="rd", name="rd")
                            nc.vector.reciprocal(rd[:, lo:hi],
                                                 pvp[64:65, lo:hi])
                            rb = precb.tile([128, 512], F32, tag="rb", name="rb")
                            nc.gpsimd.partition_broadcast(rb[0:64, lo:hi],
                                                          rd[:, lo:hi])
                            nc.vector.tensor_mul(
                                atq[po:po + 64, hp, lo:hi],
                                pvp[0:64, lo:hi], rb[0:64, lo:hi])

                    def norm_defer(hp, st, batch=batch):
                        # Chain end for a batched chunk: ACT-copies the bf16
                        # numerator into atq and the denominator into the
                        # shared rd4 collection tile; division happens in the
                        # next norm-batch unit (off the critical path).
                        if batch["rd4"] is None:
                            batch["rd4"] = prd4.tile([128, 512], F32,
                                                     tag="rd4", name="rd4")
                            nc.vector.memset(batch["rd4"][:], 1.0)
                        for half in range(2):
                            pvp = st["pvps"][half]
                            aun = paun.tile([64, 512], B16, tag="aun",
                                            name="aun")
                            nc.scalar.activation(
                                out=aun[:], in_=pvp[0:64, :], func=Copy)
                            r = 64 * (hp % 2) + 32 * half
                            nc.scalar.activation(
                                out=batch["rd4"][r:r + 1, :],
                                in_=pvp[64:65, :], func=Copy)
                            batch["rows"].append((hp, half, r, aun))

                    def emit_pv(hp, st, last, piecewise=False):
                        pj, ppt, pc0 = st["pend"].pop(0)
                        if pj == 0:
                            st["pvps"] = [
                                pspv.tile([65, 512], F32, tag="pv", name="pvpa"),
                                pspv.tile([65, 512], F32, tag="pv", name="pvpb"),
                            ]
                        for half in range(2):
                            h = 2 * hp + half
                            nc.tensor.matmul(
                                st["pvps"][half][:, pc0:512],
                                vt[:, pj, 65 * h:65 * h + 65],
                                ppt[:, half, pc0:512],
                                start=(pj == 0), stop=last,
                            )
                        # columns [128*toff, 128*toff+128) got their final PV
                        # contribution: normalize them right away so the
                        # O-projection never waits on a monolithic normalize.
                        if piecewise:
                            toff = pj - 4 * sp
                            if toff >= 0:
                                norm_piece(hp, st, 128 * toff, 128 * toff + 128)

                    def mk_step(hp, j, st=st):
                        def step_u():
                            toff = j - 4 * sp
                            c0 = 128 * toff if toff > 0 else 0
                            sm = psmm.tile([128, 2, 512], F32, tag="sm", name="sm")
                            for half in range(2):  # head 2hp+half in PE band
                                po = 64 * half
                                nc.tensor.matmul(
                                    sm[:, half, c0:512],
                                    kt[po:po + 64, hp, 128 * j:128 * (j + 1)],
                                    qt[po:po + 64, hp, q0 + c0:q0 + 512],
                                    start=True, stop=True,
                                )
                            if toff >= 0:  # diagonal: triangle add (both)
                                nc.vector.tensor_add(
                                    sm[:, :, c0:c0 + 128], sm[:, :, c0:c0 + 128],
                                    tri2_sb)
                            pt = pprob.tile([128, 2, 512], B16, tag="pt", name="pt")
                            nc.scalar.activation(
                                out=pt[:, :, c0:512], in_=sm[:, :, c0:512],
                                func=Exp, scale=0.125)
                            st["pend"].append((j, pt, c0))
                            if len(st["pend"]) > LAG:
                                emit_pv(hp, st, last=False,
                                        piecewise=hp in piecewise_norm)
                        return step_u

                    def mk_flush(hp, st=st):
                        def flush_u():
                            pw = hp in piecewise_norm
                            while st["pend"]:
                                emit_pv(hp, st, last=not st["pend"][1:],
                                        piecewise=pw)
                            if pw:
                                return
                            if batched:
                                norm_defer(hp, st)
                            else:  # normalize both heads in one go
                                norm_piece(hp, st, 0, 512)
                        return flush_u

                    for j in range(J):
                        units.append(mk_step(hp, j))
                    units.append(mk_flush(hp))
                    if batched and hp % 2 == 1:
                        units.append(mk_norm_batch())
                return units

            def o_units(sp, atq):
                units = []
                for k in range(4):
                    for oh in range(2):
                        def o_u(k=k, oh=oh):
                            ni = 4 * sp + k
                            pm = pprj.tile([128, 512], F32, tag="mm", name="pmo")
                            for uc in range(4):
                                nc.tensor.matmul(
                                    pm[:],
                                    atq[:, uc, 128 * k:128 * (k + 1)],
                                    wo_sb[:, uc, 512 * oh:512 * (oh + 1)],
                                    start=(uc == 0), stop=(uc == 3),
                                )
                            ob = pout.tile([128, 512], B16, tag="ob", name="ob")
                            nc.vector.tensor_add(
                                ob[:], pm[:], bob_sb[:, 512 * oh:512 * (oh + 1)])
                            nc.scalar.dma_start(
                                out=outr[:, ni, 512 * oh:512 * (oh + 1)], in_=ob[:])
                        units.append(o_u)
                return units

            def o_units_split(sp, atq):
                """O-proj split in two half-accumulations: the uc 0/1 part
                can run while head-pairs 2/3 are still in attention."""
                obas = {}
                ua, ub = [], []
                for k in range(4):
                    for oh in range(2):
                        def oa_u(k=k, oh=oh):
                            pm = pprj.tile([128, 512], F32, tag="mm", name="pmoa")
                            for uc in range(2):
                                nc.tensor.matmul(
                                    pm[:],
                                    atq[:, uc, 128 * k:128 * (k + 1)],
                                    wo_sb[:, uc, 512 * oh:512 * (oh + 1)],
                                    start=(uc == 0), stop=(uc == 1),
                                )
                            oba = poba.tile([128, 512], B16, tag="oba", name="oba")
                            nc.vector.tensor_add(
                                oba[:], pm[:], bob_sb[:, 512 * oh:512 * (oh + 1)])
                            obas[(k, oh)] = oba

                        def ob_u(k=k, oh=oh):
                            ni = 4 * sp + k
                            pm = pprj.tile([128, 512], F32, tag="mm", name="pmob")
                            for uc in range(2, 4):
                                nc.tensor.matmul(
                                    pm[:],
                                    atq[:, uc, 128 * k:128 * (k + 1)],
                                    wo_sb[:, uc, 512 * oh:512 * (oh + 1)],
                                    start=(uc == 2), stop=(uc == 3),
                                )
                            ob = pout.tile([128, 512], B16, tag="ob", name="ob")
                            nc.vector.tensor_add(ob[:], pm[:], obas[(k, oh)][:])
                            nc.scalar.dma_start(
                                out=outr[:, ni, 512 * oh:512 * (oh + 1)], in_=ob[:])
                        ua.append(oa_u)
                        ub.append(ob_u)
                return ua, ub

            def run_interleaved(cur, prev):
                # proportional round-robin interleave of cur and prev
                na, nb = len(cur), len(prev)
                ia = ib = 0
                while ia < na or ib < nb:
                    if ib * max(na, 1) * ratio[1] <= ia * max(nb, 1) * ratio[0] and ib < nb or ia >= na:
                        prev[ib](); ib += 1
                    else:
                        cur[ia](); ia += 1

            xss = {0: xs0, 1: xs1}
            atqs = {}
            for sp in range(1, 4):
                atqs[sp - 1] = None
            nb_pend = []
            for sp in range(4):
                cur = list(nb_pend)
                nb_pend = []
                cur += proj_units(sp)
                if sp >= 2:
                    cur = cur + o_units(sp - 2, atqs[sp - 2])
                prev = []
                if sp >= 1:
                    atqs[sp - 1] = patq.tile([128, 4, 512], B16, tag="atq",
                                             name=f"atq{sp - 1}")
                    prev = attn_units(sp - 1, atqs[sp - 1], batched=True)
                run_interleaved(cur, prev)
            # phase 4: attention chunk 3; O(2) rides the first two chains,
            # O(3)'s uc0/1 half rides the last two, its uc2/3 half drains.
            atqs[3] = patq.tile([128, 4, 512], B16, tag="atq", name="atq3")
            o3a, o3b = o_units_split(3, atqs[3])
            run_interleaved(nb_pend + o_units(2, atqs[2]),
                            attn_units(3, atqs[3], hps=(0, 1), batched=True))
            run_interleaved(o3a,
                            attn_units(3, atqs[3], hps=(2, 3),
                                       piecewise_norm=(3,)))
            for u in o3b:
                u()

    nc.compile()
    return nc


_NC_CACHE = {}


def _get_nc(repeat=1, **kw):
    key = (repeat, tuple(sorted(kw.items())))
    if key not in _NC_CACHE:
        _NC_CACHE[key] = _build(repeat, **kw)
    return _NC_CACHE[key]


def _host_inputs(x, Wq, bq, Wk, bk, Wv, bv, Wo, bo):
    """Build the 8 per-core input maps."""
    import ml_dtypes
    f32 = np.float32
    B16 = ml_dtypes.bfloat16

    def rnd(a):
        return np.ascontiguousarray(a, dtype=f32).astype(B16)

    r = np.arange(128)[:, None]
    c = np.arange(128)[None, :]
    mask1 = np.where(r <= c, f32(0.0), f32(-1e4)).astype(f32)
    mask = np.concatenate([mask1, mask1], axis=1)

    in_maps = []
    for core in range(8):
        bi, hg = core // 2, core % 2
        hsl = slice(512 * hg, 512 * (hg + 1))
        # xT swizzled: [ns, p, kc, col] = x[bi].T[kc*128+p, 256*ns+col]
        xTl = rnd(np.ascontiguousarray(
            x[bi].T.reshape(8, 128, 8, 256).transpose(2, 1, 0, 3)))
        wqkTl = rnd(np.ascontiguousarray(
            np.concatenate([Wq[hsl].T, Wk[hsl].T], axis=1).reshape(8, 128, 1024)
            .transpose(1, 0, 2)))
        wvTl = rnd(np.ascontiguousarray(
            Wv[hsl].T.reshape(8, 128, 512).transpose(1, 0, 2)))
        woTl = rnd(np.ascontiguousarray(
            Wo[:, hsl].T.reshape(4, 128, 1024).transpose(1, 0, 2)))
        bq_l, bk_l = bq[hsl], bk[hsl]
        bqk_t = np.stack(
            [bq_l[128 * i:128 * (i + 1)] for i in range(4)]
            + [bk_l[128 * i:128 * (i + 1)] for i in range(4)]
            + [np.ones(128, dtype=f32)] * 8, axis=1
        ).astype(f32)
        bvb_t = np.broadcast_to(bv[hsl].astype(f32), (128, 512)).copy()
        if hg == 0:
            bob_t = np.broadcast_to(bo.astype(f32), (128, 1024)).copy()
        else:
            bob_t = np.zeros((128, 1024), dtype=f32)
        in_maps.append({
            "xT": xTl, "wqkT": wqkTl, "wvT": wvTl, "woT": woTl,
            "bqk": bqk_t, "bvb": bvb_t, "bob": bob_t, "maskt": mask,
        })
    return in_maps


def kernel(x, Wq, bq, Wk, bk, Wv, bv, Wo, bo):
    from concourse.bass_utils import run_bass_kernel_spmd

    x = np.asarray(x); Wq = np.asarray(Wq); bq = np.asarray(bq)
    Wk = np.asarray(Wk); bk = np.asarray(bk); Wv = np.asarray(Wv)
    bv = np.asarray(bv); Wo = np.asarray(Wo); bo = np.asarray(bo)

    nc = _get_nc()
    in_maps = _host_inputs(x, Wq, bq, Wk, bk, Wv, bv, Wo, bo)
    r = run_bass_kernel_spmd(nc, in_maps, list(range(8)))

    out = np.empty((4, 2048, 1024), dtype=np.float32)
    for bi in range(4):
        out[bi] = (r.results[2 * bi]["out"].astype(np.float32)
                   + r.results[2 * bi + 1]["out"].astype(np.float32))
    return out


# revision 43
# speedup vs baseline: 1.1317x; 1.0793x over previous
"""Causal self-attention (b=4, s=2048, d=1024, h=16, hd=64) on 8 trn2 cores.

Sharding: (batch, head-group) — core c handles batch c//2 and heads
[8*(c%2), 8*(c%2)+8) (Megatron column-parallel QKV + row-parallel O).
Each core returns a partial (2048, 1024) bf16 output for its batch; the
host upcasts and sums the two partials per batch (the row-parallel
reduce of the Megatron pattern, done as part of unsharding).

All matmul operands are bf16 (full PE rate like fp32r, half the
DMA/SBUF traffic, and no N>=256 full-rate constraint); accumulation is
fp32 in PSUM, biases fp32.

Per-core device program (layouts chosen so NO on-chip transposes are
needed):
    xT (1024,2048) = x[b].T feeds both Q^T/K^T (as moving operand) and
    V (as stationary operand).  Q^T/K^T stored [o=512 part-dims, n];
    V stored [n part, o free] with a ones column per head so the softmax
    denominator falls out of the PV matmul (M=65).  probs kept
    TRANSPOSED [kv, q]: softmax needs no max-subtraction (scores bounded
    ~|3|), the causal mask is additive (-1e4 pre-exp, exp underflows to
    0), and attn^T [u, n] is directly the stationary operand of the
    O-projection.  Causality: fully-masked kv-chunks are skipped
    entirely; on diagonal chunks only the live column range is computed.

    Schedule: 5 phases; phase p emits the projections of x-slabs
    (2p, 2p+1) interleaved with the attention of q-chunk p-1; the
    O-projection of chunk p-2 rides along one phase later so it never
    waits on the softmax normalize.  Attention runs as sequential
    head-pair chains whose PV matmuls lag the S_T/exp stream by `lag`
    steps, so the in-order PE never waits on the ACT exp.  Scores for a
    head pair land in ONE merged [128,2,512] PSUM tile, so mask-add and
    exp are single instructions over both heads.  PSUM: 2 merged score
    tiles (4 banks) + 2 projection accumulators (2 banks) + 2 PV
    accumulator banks.

    Softmax normalize: nc.vector.reciprocal is ~2.8us on HW (multi-pass)
    and blocks the in-order DVE queue, so denominators are collected
    across two chains into one [128,512] tile (rows 0/32/64/96 — the
    only partition bases single-row ACT/DVE ops support) via cheap ACT
    copies, reciprocal'd in ONE DVE op, then broadcast+multiplied into
    the (ACT-copied) bf16 numerators off the critical path.  The last
    chunk's final chain instead normalizes piecewise per 128-column
    group as its diagonal PVs complete, so the O-projection tail starts
    immediately.  (gpsimd partition_broadcast only works from/to
    partition base 0 on HW; ACT stages rows there first.)

    The final chunk's O-projection is split into uc{0,1} and uc{2,3}
    half-accumulations so its first half overlaps the last two
    attention chains.

    Measured (NTFF, single dispatch, core 0): ~331 us at throttle-util
    0.85 (~282 us unthrottled-equivalent) vs 577 us (at util 0.60;
    ~349 us equivalent) for the fp32r predecessor.  Cost-model
    prediction (no throttle): ~270 us.  rel err vs fp32 reference:
    4.1e-3 (threshold 2e-2).
"""
from contextlib import ExitStack

import numpy as np

MM_MODE = "bf16"  # kept for test.py compat; only bf16 path exists


def _build(repeat=1, ratio=(2, 1), lag=3):
    import concourse.tile as tile
    from concourse import bacc, mybir

    dt = mybir.dt
    F32 = dt.float32
    B16 = dt.bfloat16
    Exp = mybir.ActivationFunctionType.Exp
    Copy = mybir.ActivationFunctionType.Copy
    Identity = mybir.ActivationFunctionType.Identity

    nc = bacc.Bacc("TRN2", target_bir_lowering=False, debug=False, num_devices=8)

    xT = nc.dram_tensor("xT", [8, 128, 8, 256], B16, kind="ExternalInput").ap()
    wqkT = nc.dram_tensor("wqkT", [128, 8, 1024], B16, kind="ExternalInput").ap()
    wvT = nc.dram_tensor("wvT", [128, 8, 512], B16, kind="ExternalInput").ap()
    woT = nc.dram_tensor("woT", [128, 4, 1024], B16, kind="ExternalInput").ap()
    bqk = nc.dram_tensor("bqk", [128, 16], F32, kind="ExternalInput").ap()
    bvb = nc.dram_tensor("bvb", [128, 512], F32, kind="ExternalInput").ap()
    bob = nc.dram_tensor("bob", [128, 1024], F32, kind="ExternalInput").ap()
    maskt = nc.dram_tensor("maskt", [128, 256], F32, kind="ExternalInput").ap()
    out = nc.dram_tensor("out", [2048, 1024], B16, kind="ExternalOutput").ap()

    outr = out.rearrange("(nc p) o -> p nc o", p=128)    # [128, 16, 1024]

    with tile.TileContext(nc) as tc, ExitStack() as ctx:
        big = ctx.enter_context(tc.tile_pool(name="big", bufs=1))
        pqt = ctx.enter_context(tc.tile_pool(name="pqt", bufs=1))
        pkt = ctx.enter_context(tc.tile_pool(name="pkt", bufs=1))
        pv = ctx.enter_context(tc.tile_pool(name="pv", bufs=1))
        pxs = ctx.enter_context(tc.tile_pool(name="pxs", bufs=3))
        pprob = ctx.enter_context(tc.tile_pool(name="pprob", bufs=4))
        precb = ctx.enter_context(tc.tile_pool(name="precb", bufs=1))
        prd = ctx.enter_context(tc.tile_pool(name="prd", bufs=1))
        prd4 = ctx.enter_context(tc.tile_pool(name="prd4", bufs=2))
        pone = ctx.enter_context(tc.tile_pool(name="pone", bufs=1))
        pout = ctx.enter_context(tc.tile_pool(name="pout", bufs=2))
        poba = ctx.enter_context(tc.tile_pool(name="poba", bufs=8))
        paun = ctx.enter_context(tc.tile_pool(name="paun", bufs=4))
        patq = ctx.enter_context(tc.tile_pool(name="patq", bufs=2))
        psmm = ctx.enter_context(tc.tile_pool(name="psmm", bufs=2, space="PSUM"))
        pprj = ctx.enter_context(tc.tile_pool(name="pprj", bufs=2, space="PSUM"))
        pspv = ctx.enter_context(tc.tile_pool(name="pspv", bufs=2, space="PSUM"))

        # ---- constants (one merged tile: bqk | ones8 | bvb | bob | mask2) ----
        const_sb = pone.tile([128, 1808], F32, tag="const")
        bqk_sb = const_sb[:, 0:8]
        ones8_sb = const_sb[:, 8:16]
        bvb_sb = const_sb[:, 16:528]
        bob_sb = const_sb[:, 528:1552]
        tri2_sb = const_sb[:, 1552:1808].rearrange("p (two e) -> p two e", e=128)

        for rep in range(repeat):
            # prefetch the first two x slabs so projections start ASAP
            xs0 = pxs.tile([128, 8, 256], B16, tag="xs")
            nc.sync.dma_start(out=xs0[:, 0:4], in_=xT[0, :, 0:4])
            # ---- weights ----
            wv_sb = big.tile([128, 8, 512], B16, tag="bigB")
            nc.sync.dma_start(out=wv_sb[:, 0:4], in_=wvT[:, 0:4])
            nc.sync.dma_start(out=xs0[:, 4:8], in_=xT[0, :, 4:8])
            nc.sync.dma_start(out=wv_sb[:, 4:8], in_=wvT[:, 4:8])
            if rep == 0:
                nc.sync.dma_start(out=const_sb[:, 0:16], in_=bqk)
                nc.sync.dma_start(out=bvb_sb, in_=bvb)
            xs1 = pxs.tile([128, 8, 256], B16, tag="xs", name="xs1")
            nc.sync.dma_start(out=xs1[:, 0:4], in_=xT[1, :, 0:4])
            nc.sync.dma_start(out=xs1[:, 4:8], in_=xT[1, :, 4:8])
            wqk_sb = big.tile([128, 8, 1024], B16, tag="bigA")
            for kc in range(8):
                nc.sync.dma_start(out=wqk_sb[:, kc], in_=wqkT[:, kc])
            if rep == 0:
                nc.sync.dma_start(out=bob_sb, in_=bob)
                nc.sync.dma_start(out=const_sb[:, 1552:1808], in_=maskt)
            wo_sb = big.tile([128, 4, 1024], B16, tag="bigC")
            nc.sync.dma_start(out=wo_sb[:], in_=woT)

            # ---- persistent activations ----
            qt = pqt.tile([128, 4, 2048], B16)   # Q^T: u-dim on partitions
            kt = pkt.tile([128, 4, 2048], B16)   # K^T
            vt = pv.tile([128, 16, 520], B16)    # V: [n part, 8*(64+ones)]

            # 5 phases: phase p emits projections for slabs (2p, 2p+1)
            # INTERLEAVED with the attention of q-chunk p-1; the O-proj of
            # chunk p-2 rides along one phase later (its normalize is long
            # done by then, so it never stalls the PE).
            def proj_units(sp):
                units = []

                def mk_dma(ns):
                    def dma_u():
                        xs = pxs.tile([128, 8, 256], B16, tag="xs", name=f"xs{ns}")
                        nc.sync.dma_start(out=xs[:, 0:4], in_=xT[ns, :, 0:4])
                        nc.sync.dma_start(out=xs[:, 4:8], in_=xT[ns, :, 4:8])
                        xss[ns] = xs
                    return dma_u

                def mk_v(ns, nn):
                    def v_u():
                        ni = 2 * ns + nn
                        pmv = pprj.tile([128, 512], F32, tag="mm", name="pmv")
                        for kc in range(8):
                            nc.tensor.matmul(
                                pmv[:],
                                xss[ns][:, kc, 128 * nn:128 * (nn + 1)],
                                wv_sb[:, kc, :],
                                start=(kc == 0), stop=(kc == 7),
                            )
                        vslab = vt[:, ni, :].rearrange("p (h e) -> p h e", e=65)
                        nc.vector.tensor_copy(out=vslab[:, :, 64], in_=ones8_sb)
                        nc.vector.tensor_add(
                            vslab[:, :, 0:64],
                            pmv[:].rearrange("p (h e) -> p h e", e=64),
                            bvb_sb.rearrange("p (h e) -> p h e", e=64),
                        )
                    return v_u

                def mk_qk(ns, oc):
                    def qk_u():
                        pm = pprj.tile([128, 256], F32, tag="mm", name="pmqk")
                        for kc in range(8):
                            nc.tensor.matmul(
                                pm[:],
                                wqk_sb[:, kc, 128 * oc:128 * (oc + 1)],
                                xss[ns][:, kc, :],
                                start=(kc == 0), stop=(kc == 7),
                            )
                        dest = qt if oc < 4 else kt
                        nc.vector.tensor_scalar_add(
                            dest[:, oc % 4, 256 * ns:256 * (ns + 1)], pm[:],
                            bqk_sb[:, oc:oc + 1],
                        )
                    return qk_u

                # x slabs for THIS phase were prefetched last phase; here we
                # prefetch the next phase's two slabs.
                if sp == 0:
                    # V first: needs only xs+wv (the first DMAs to land);
                    # the QK units then overlap the wqk stream.
                    units.extend([mk_v(0, 0), mk_v(0, 1), mk_v(1, 0), mk_v(1, 1)])
                    units.append(mk_dma(2))
                    units.extend([mk_qk(0, oc) for oc in range(8)])
                    units.append(mk_dma(3))
                    units.extend([mk_qk(1, oc) for oc in range(8)])
                else:
                    if sp < 3:
                        units.append(mk_dma(2 * sp + 2))
                    units.extend([mk_qk(2 * sp, oc) for oc in range(8)])
                    if sp < 3:
                        units.append(mk_dma(2 * sp + 3))
                    units.extend([mk_qk(2 * sp + 1, oc) for oc in range(8)])
                    units.extend([mk_v(2 * sp, 0), mk_v(2 * sp, 1),
                                  mk_v(2 * sp + 1, 0), mk_v(2 * sp + 1, 1)])
                return units

            def attn_units(sp, atq, hps=(0, 1, 2, 3), piecewise_norm=(),
                           batched=False, nb_sink=None):
                q0 = 512 * sp
                J = 4 * (sp + 1)
                LAG = lag   # PV of step j is emitted inside step j+LAG's unit
                units = []
                # Sequential head-pair chains; each chain's PV runs LAG steps
                # behind its S_T/exp so the PE never waits on the exp.
                batch = {"rd4": None, "rows": []}

                def mk_norm_batch(batch=batch):
                    # One DVE reciprocal for 4 collected denominators (at
                    # partition bases 0/32/64/96), then broadcast + in-place
                    # multiply of the already-copied bf16 numerators in atq.
                    def nb_u():
                        rd4 = batch["rd4"]
                        rr4 = prd.tile([128, 512], F32, tag="rr4", name="rr4")
                        nc.vector.reciprocal(rr4[:], rd4[:])
                        for (hp, half, r, aun) in batch["rows"]:
                            po = 64 * half
                            # partition_broadcast only works from/to base 0
                            # on HW: stage the row at base 0 via ACT first.
                            st0 = prd.tile([1, 512], F32, tag="st", name="st")
                            nc.scalar.activation(out=st0[:], in_=rr4[r:r + 1, :],
                                                 func=Copy)
                            rb = precb.tile([128, 512], F32, tag="rb", name="rb")
                            nc.gpsimd.partition_broadcast(rb[0:64, :], st0[:])
                            nc.vector.tensor_mul(
                                atq[po:po + 64, hp, :],
                                aun[0:64, :], rb[0:64, :])
                        batch["rd4"] = None
                        batch["rows"] = []
                    return nb_u

                for hp in hps:
                    st = {"pvps": None, "pend": []}

                    def norm_piece(hp, st, lo, hi):
                        for half in range(2):
                            po = 64 * half
                            pvp = st["pvps"][half]
                            rd = prd.tile([1, 512], F32, tag="rd", name="rd")
                            nc.vector.reciprocal(rd[:, lo:hi],
                                                 pvp[64:65, lo:hi])
                            rb = precb.tile([128, 512], F32, tag="rb", name="rb")
                            nc.gpsimd.partition_broadcast(rb[0:64, lo:hi],
                                                          rd[:, lo:hi])
                            nc.vector.tensor_mul(
                                atq[po:po + 64, hp, lo:hi],
                                pvp[0:64, lo:hi], rb[0:64, lo:hi])

                    def norm_defer(hp, st, batch=batch):
                        # Chain end for a batched chunk: ACT-copies the bf16
                        # numerator into atq and the denominator into the
                        # shared rd4 collection tile; division happens in the
                        # next norm-batch unit (off the critical path).
                        if batch["rd4"] is None:
                            batch["rd4"] = prd4.tile([128, 512], F32,
                                                     tag="rd4", name="rd4")
                            nc.vector.memset(batch["rd4"][:], 1.0)
                        for half in range(2):
                            pvp = st["pvps"][half]
                            aun = paun.tile([64, 512], B16, tag="aun",
                                            name="aun")
                            nc.scalar.activation(
                                out=aun[:], in_=pvp[0:64, :], func=Copy)
                            r = 64 * (hp % 2) + 32 * half
                            nc.scalar.activation(
                                out=batch["rd4"][r:r + 1, :],
                                in_=pvp[64:65, :], func=Copy)
                            batch["rows"].append((hp, half, r, aun))

                    def emit_pv(hp, st, last, piecewise=False):
                        pj, ppt, pc0 = st["pend"].pop(0)
                        if pj == 0:
                            st["pvps"] = [
                                pspv.tile([65, 512], F32, tag="pv", name="pvpa"),
                                pspv.tile([65, 512], F32, tag="pv", name="pvpb"),
                            ]
                        for half in range(2):
                            h = 2 * hp + half
                            nc.tensor.matmul(
                                st["pvps"][half][:, pc0:512],
                                vt[:, pj, 65 * h:65 * h + 65],
                                ppt[:, half, pc0:512],
                                start=(pj == 0), stop=last,
                            )
                        # columns [128*toff, 128*toff+128) got their final PV
                        # contribution: normalize them right away so the
                        # O-projection never waits on a monolithic normalize.
                        if piecewise:
                            toff = pj - 4 * sp
                            if toff >= 0:
                                norm_piece(hp, st, 128 * toff, 128 * toff + 128)

                    def mk_step(hp, j, st=st):
                        def step_u():
                            toff = j - 4 * sp
                            c0 = 128 * toff if toff > 0 else 0
                            sm = psmm.tile([128, 2, 512], F32, tag="sm", name="sm")
                            for half in range(2):  # head 2hp+half in PE band
                                po = 64 * half
                                nc.tensor.matmul(
                                    sm[:, half, c0:512],
                                    kt[po:po + 64, hp, 128 * j:128 * (j + 1)],
                                    qt[po:po + 64, hp, q0 + c0:q0 + 512],
                                    start=True, stop=True,
                                )
                            if toff >= 0:  # diagonal: triangle add (both)
                                nc.vector.tensor_add(
                                    sm[:, :, c0:c0 + 128], sm[:, :, c0:c0 + 128],
                                    tri2_sb)
                            pt = pprob.tile([128, 2, 512], B16, tag="pt", name="pt")
                            nc.scalar.activation(
                                out=pt[:, :, c0:512], in_=sm[:, :, c0:512],
                                func=Exp, scale=0.125)
                            st["pend"].append((j, pt, c0))
                            if len(st["pend"]) > LAG:
                                emit_pv(hp, st, last=False,
                                        piecewise=hp in piecewise_norm)
                        return step_u

                    def mk_flush(hp, st=st):
                        def flush_u():
                            pw = hp in piecewise_norm
                            while st["pend"]:
                                emit_pv(hp, st, last=not st["pend"][1:],
                                        piecewise=pw)
                            if pw:
                                return
                            if batched:
                                norm_defer(hp, st)
                            else:  # normalize both heads in one go
                                norm_piece(hp, st, 0, 512)
                        return flush_u

                    for j in range(J):
                        units.append(mk_step(hp, j))
                    units.append(mk_flush(hp))
                    if batched and hp % 2 == 1:
                        nb = mk_norm_batch()
                        if hp == 3 and nb_sink is not None:
                            nb_sink.append(nb)   # run early next phase
                        else:
                            units.append(nb)
                return units

            def o_units(sp, atq):
                units = []
                for k in range(4):
                    for oh in range(2):
                        def o_u(k=k, oh=oh):
                            ni = 4 * sp + k
                            pm = pprj.tile([128, 512], F32, tag="mm", name="pmo")
                            for uc in range(4):
                                nc.tensor.matmul(
                                    pm[:],
                                    atq[:, uc, 128 * k:128 * (k + 1)],
                                    wo_sb[:, uc, 512 * oh:512 * (oh + 1)],
                                    start=(uc == 0), stop=(uc == 3),
                                )
                            ob = pout.tile([128, 512], B16, tag="ob", name="ob")
                            nc.vector.tensor_add(
                                ob[:], pm[:], bob_sb[:, 512 * oh:512 * (oh + 1)])
                            nc.scalar.dma_start(
                                out=outr[:, ni, 512 * oh:512 * (oh + 1)], in_=ob[:])
                        units.append(o_u)
                return units

            def o_units_split(sp, atq):
                """O-proj split in two half-accumulations: the uc 0/1 part
                can run while head-pairs 2/3 are still in attention."""
                obas = {}
                ua, ub = [], []
                for k in range(4):
                    for oh in range(2):
                        def oa_u(k=k, oh=oh):
                            pm = pprj.tile([128, 512], F32, tag="mm", name="pmoa")
                            for uc in range(2):
                                nc.tensor.matmul(
                                    pm[:],
                                    atq[:, uc, 128 * k:128 * (k + 1)],
                                    wo_sb[:, uc, 512 * oh:512 * (oh + 1)],
                                    start=(uc == 0), stop=(uc == 1),
                                )
                            oba = poba.tile([128, 512], B16, tag="oba", name="oba")
                            nc.vector.tensor_add(
                                oba[:], pm[:], bob_sb[:, 512 * oh:512 * (oh + 1)])
                            obas[(k, oh)] = oba

                        def ob_u(k=k, oh=oh):
                            ni = 4 * sp + k
                            pm = pprj.tile([128, 512], F32, tag="mm", name="pmob")
                            for uc in range(2, 4):
                                nc.tensor.matmul(
                                    pm[:],
                                    atq[:, uc, 128 * k:128 * (k + 1)],
                                    wo_sb[:, uc, 512 * oh:512 * (oh + 1)],
                                    start=(uc == 2), stop=(uc == 3),
                                )
                            ob = pout.tile([128, 512], B16, tag="ob", name="ob")
                            nc.vector.tensor_add(ob[:], pm[:], obas[(k, oh)][:])
                            nc.scalar.dma_start(
                                out=outr[:, ni, 512 * oh:512 * (oh + 1)], in_=ob[:])
                        ua.append(oa_u)
                        ub.append(ob_u)
                return ua, ub

            def run_interleaved(cur, prev):
                # proportional round-robin interleave of cur and prev
                na, nb = len(cur), len(prev)
                ia = ib = 0
                while ia < na or ib < nb:
                    if ib * max(na, 1) * ratio[1] <= ia * max(nb, 1) * ratio[0] and ib < nb or ia >= na:
                        prev[ib](); ib += 1
                    else:
                        cur[ia](); ia += 1

            xss = {0: xs0, 1: xs1}
            atqs = {}
            for sp in range(1, 4):
                atqs[sp - 1] = None
            nb_pend = []
            for sp in range(4):
                cur = list(nb_pend)
                nb_pend = []
                cur += proj_units(sp)
                if sp >= 2:
                    cur = cur + o_units(sp - 2, atqs[sp - 2])
                prev = []
                if sp >= 1:
                    atqs[sp - 1] = patq.tile([128, 4, 512], B16, tag="atq",
                                             name=f"atq{sp - 1}")
                    prev = attn_units(sp - 1, atqs[sp - 1], batched=True,
                                      nb_sink=nb_pend)
                run_interleaved(cur, prev)
            # phase 4: attention chunk 3; O(2) rides the first two chains,
            # O(3)'s uc0/1 half rides the last two, its uc2/3 half drains.
            atqs[3] = patq.tile([128, 4, 512], B16, tag="atq", name="atq3")
            o3a, o3b = o_units_split(3, atqs[3])
            run_interleaved(nb_pend + o_units(2, atqs[2]),
                            attn_units(3, atqs[3], hps=(0, 1), batched=True))
            run_interleaved(o3a,
                            attn_units(3, atqs[3], hps=(2, 3),
                                       piecewise_norm=(3,)))
            for u in o3b:
                u()

    nc.compile()
    return nc


_NC_CACHE = {}


def _get_nc(repeat=1, **kw):
    key = (repeat, tuple(sorted(kw.items())))
    if key not in _NC_CACHE:
        _NC_CACHE[key] = _build(repeat, **kw)
    return _NC_CACHE[key]


def _host_inputs(x, Wq, bq, Wk, bk, Wv, bv, Wo, bo):
    """Build the 8 per-core input maps."""
    import ml_dtypes
    f32 = np.float32
    B16 = ml_dtypes.bfloat16

    def rnd(a):
        return np.ascontiguousarray(a, dtype=f32).astype(B16)

    r = np.arange(128)[:, None]
    c = np.arange(128)[None, :]
    mask1 = np.where(r <= c, f32(0.0), f32(-1e4)).astype(f32)
    mask = np.concatenate([mask1, mask1], axis=1)

    in_maps = []
    for core in range(8):
        bi, hg = core // 2, core % 2
        hsl = slice(512 * hg, 512 * (hg + 1))
        # xT swizzled: [ns, p, kc, col] = x[bi].T[kc*128+p, 256*ns+col]
        xTl = rnd(np.ascontiguousarray(
            x[bi].T.reshape(8, 128, 8, 256).transpose(2, 1, 0, 3)))
        wqkTl = rnd(np.ascontiguousarray(
            np.concatenate([Wq[hsl].T, Wk[hsl].T], axis=1).reshape(8, 128, 1024)
            .transpose(1, 0, 2)))
        wvTl = rnd(np.ascontiguousarray(
            Wv[hsl].T.reshape(8, 128, 512).transpose(1, 0, 2)))
        woTl = rnd(np.ascontiguousarray(
            Wo[:, hsl].T.reshape(4, 128, 1024).transpose(1, 0, 2)))
        bq_l, bk_l = bq[hsl], bk[hsl]
        bqk_t = np.stack(
            [bq_l[128 * i:128 * (i + 1)] for i in range(4)]
            + [bk_l[128 * i:128 * (i + 1)] for i in range(4)]
            + [np.ones(128, dtype=f32)] * 8, axis=1
        ).astype(f32)
        bvb_t = np.broadcast_to(bv[hsl].astype(f32), (128, 512)).copy()
        if hg == 0:
            bob_t = np.broadcast_to(bo.astype(f32), (128, 1024)).copy()
        else:
            bob_t = np.zeros((128, 1024), dtype=f32)
        in_maps.append({
            "xT": xTl, "wqkT": wqkTl, "wvT": wvTl, "woT": woTl,
            "bqk": bqk_t, "bvb": bvb_t, "bob": bob_t, "maskt": mask,
        })
    return in_maps


def kernel(x, Wq, bq, Wk, bk, Wv, bv, Wo, bo):
    from concourse.bass_utils import run_bass_kernel_spmd

    x = np.asarray(x); Wq = np.asarray(Wq); bq = np.asarray(bq)
    Wk = np.asarray(Wk); bk = np.asarray(bk); Wv = np.asarray(Wv)
    bv = np.asarray(bv); Wo = np.asarray(Wo); bo = np.asarray(bo)

    nc = _get_nc()
    in_maps = _host_inputs(x, Wq, bq, Wk, bk, Wv, bv, Wo, bo)
    r = run_bass_kernel_spmd(nc, in_maps, list(range(8)))

    out = np.empty((4, 2048, 1024), dtype=np.float32)
    for bi in range(4):
        out[bi] = (r.results[2 * bi]["out"].astype(np.float32)
                   + r.results[2 * bi + 1]["out"].astype(np.float32))
    return out


# revision 44
# speedup vs baseline: 1.1606x; 1.0256x over previous
"""Causal self-attention (b=4, s=2048, d=1024, h=16, hd=64) on 8 trn2 cores.

Sharding: (batch, head-group) — core c handles batch c//2 and heads
[8*(c%2), 8*(c%2)+8) (Megatron column-parallel QKV + row-parallel O).
Each core returns a partial (2048, 1024) bf16 output for its batch; the
host upcasts and sums the two partials per batch (the row-parallel
reduce of the Megatron pattern, done as part of unsharding).

All matmul operands are bf16 (full PE rate like fp32r, half the
DMA/SBUF traffic, and no N>=256 full-rate constraint); accumulation is
fp32 in PSUM, biases fp32.

Per-core device program (layouts chosen so NO on-chip transposes are
needed):
    xT (1024,2048) = x[b].T feeds both Q^T/K^T (as moving operand) and
    V (as stationary operand).  Q^T/K^T stored [o=512 part-dims, n];
    V stored [n part, o free] with a ones column per head so the softmax
    denominator falls out of the PV matmul (M=65).  probs kept
    TRANSPOSED [kv, q]: softmax needs no max-subtraction (scores bounded
    ~|3|), the causal mask is additive (-1e4 pre-exp, exp underflows to
    0), and attn^T [u, n] is directly the stationary operand of the
    O-projection.  Causality: fully-masked kv-chunks are skipped
    entirely; on diagonal chunks only the live column range is computed.

    Schedule: 5 phases; phase p emits the projections of x-slabs
    (2p, 2p+1) interleaved with the attention of q-chunk p-1; the
    O-projection of chunk p-2 rides along one phase later so it never
    waits on the softmax normalize.  Attention runs as sequential
    head-pair chains whose PV matmuls lag the S_T/exp stream by `lag`
    steps, so the in-order PE never waits on the ACT exp.  Scores for a
    head pair land in ONE merged [128,2,512] PSUM tile, so mask-add and
    exp are single instructions over both heads.  PSUM: 2 merged score
    tiles (4 banks) + 2 projection accumulators (2 banks) + 2 PV
    accumulator banks.

    Softmax normalize: nc.vector.reciprocal is ~2.8us on HW (multi-pass)
    and blocks the in-order DVE queue, so denominators are collected
    across two chains into one [128,512] tile (rows 0/32/64/96 — the
    only partition bases single-row ACT/DVE ops support) via cheap ACT
    copies, reciprocal'd in ONE DVE op, then broadcast+multiplied into
    the (ACT-copied) bf16 numerators off the critical path.  The last
    chunk's final chain instead normalizes piecewise per 128-column
    group as its diagonal PVs complete, so the O-projection tail starts
    immediately.  (gpsimd partition_broadcast only works from/to
    partition base 0 on HW; ACT stages rows there first.)

    The final chunk's O-projection is split into uc{0,1} and uc{2,3}
    half-accumulations so its first half overlaps the last two
    attention chains.

    Measured (NTFF, single dispatch, core 0): ~331 us at throttle-util
    0.85 (~282 us unthrottled-equivalent) vs 577 us (at util 0.60;
    ~349 us equivalent) for the fp32r predecessor.  Cost-model
    prediction (no throttle): ~270 us.  rel err vs fp32 reference:
    4.1e-3 (threshold 2e-2).
"""
from contextlib import ExitStack

import numpy as np

MM_MODE = "bf16"  # kept for test.py compat; only bf16 path exists


def _build(repeat=1, ratio=(2, 1), lag=3):
    import concourse.tile as tile
    from concourse import bacc, mybir

    dt = mybir.dt
    F32 = dt.float32
    B16 = dt.bfloat16
    Exp = mybir.ActivationFunctionType.Exp
    Copy = mybir.ActivationFunctionType.Copy
    Identity = mybir.ActivationFunctionType.Identity

    nc = bacc.Bacc("TRN2", target_bir_lowering=False, debug=False, num_devices=8)

    xT = nc.dram_tensor("xT", [8, 128, 8, 256], B16, kind="ExternalInput").ap()
    wqkT = nc.dram_tensor("wqkT", [128, 8, 1024], B16, kind="ExternalInput").ap()
    wvT = nc.dram_tensor("wvT", [128, 8, 512], B16, kind="ExternalInput").ap()
    woT = nc.dram_tensor("woT", [128, 4, 1024], B16, kind="ExternalInput").ap()
    bqk = nc.dram_tensor("bqk", [128, 16], F32, kind="ExternalInput").ap()
    bvb = nc.dram_tensor("bvb", [128, 512], F32, kind="ExternalInput").ap()
    bob = nc.dram_tensor("bob", [128, 1024], F32, kind="ExternalInput").ap()
    maskt = nc.dram_tensor("maskt", [128, 256], F32, kind="ExternalInput").ap()
    out = nc.dram_tensor("out", [2048, 1024], B16, kind="ExternalOutput").ap()

    outr = out.rearrange("(nc p) o -> p nc o", p=128)    # [128, 16, 1024]

    with tile.TileContext(nc) as tc, ExitStack() as ctx:
        big = ctx.enter_context(tc.tile_pool(name="big", bufs=1))
        pqt = ctx.enter_context(tc.tile_pool(name="pqt", bufs=1))
        pkt = ctx.enter_context(tc.tile_pool(name="pkt", bufs=1))
        pv = ctx.enter_context(tc.tile_pool(name="pv", bufs=1))
        pxs = ctx.enter_context(tc.tile_pool(name="pxs", bufs=3))
        pprob = ctx.enter_context(tc.tile_pool(name="pprob", bufs=4))
        precb = ctx.enter_context(tc.tile_pool(name="precb", bufs=1))
        prd = ctx.enter_context(tc.tile_pool(name="prd", bufs=1))
        prd4 = ctx.enter_context(tc.tile_pool(name="prd4", bufs=2))
        pone = ctx.enter_context(tc.tile_pool(name="pone", bufs=1))
        pout = ctx.enter_context(tc.tile_pool(name="pout", bufs=2))
        poba = ctx.enter_context(tc.tile_pool(name="poba", bufs=8))
        paun = ctx.enter_context(tc.tile_pool(name="paun", bufs=4))
        patq = ctx.enter_context(tc.tile_pool(name="patq", bufs=2))
        psmm = ctx.enter_context(tc.tile_pool(name="psmm", bufs=2, space="PSUM"))
        pprj = ctx.enter_context(tc.tile_pool(name="pprj", bufs=2, space="PSUM"))
        pspv = ctx.enter_context(tc.tile_pool(name="pspv", bufs=2, space="PSUM"))

        # ---- constants (one merged tile: bqk | ones8 | bvb | bob | mask2) ----
        const_sb = pone.tile([128, 1808], F32, tag="const")
        bqk_sb = const_sb[:, 0:8]
        ones8_sb = const_sb[:, 8:16]
        bvb_sb = const_sb[:, 16:528]
        bob_sb = const_sb[:, 528:1552]
        tri2_sb = const_sb[:, 1552:1808].rearrange("p (two e) -> p two e", e=128)

        for rep in range(repeat):
            # prefetch the first two x slabs so projections start ASAP
            xs0 = pxs.tile([128, 8, 256], B16, tag="xs")
            nc.sync.dma_start(out=xs0[:, 0:4], in_=xT[0, :, 0:4])
            # ---- weights ----
            wv_sb = big.tile([128, 8, 512], B16, tag="bigB")
            nc.sync.dma_start(out=wv_sb[:, 0:4], in_=wvT[:, 0:4])
            nc.sync.dma_start(out=xs0[:, 4:8], in_=xT[0, :, 4:8])
            nc.sync.dma_start(out=wv_sb[:, 4:8], in_=wvT[:, 4:8])
            if rep == 0:
                nc.sync.dma_start(out=const_sb[:, 0:16], in_=bqk)
                nc.sync.dma_start(out=bvb_sb, in_=bvb)
            xs1 = pxs.tile([128, 8, 256], B16, tag="xs", name="xs1")
            nc.sync.dma_start(out=xs1[:, 0:4], in_=xT[1, :, 0:4])
            nc.sync.dma_start(out=xs1[:, 4:8], in_=xT[1, :, 4:8])
            wqk_sb = big.tile([128, 8, 1024], B16, tag="bigA")
            for kc in range(8):
                nc.sync.dma_start(out=wqk_sb[:, kc], in_=wqkT[:, kc])
            if rep == 0:
                nc.sync.dma_start(out=bob_sb, in_=bob)
                nc.sync.dma_start(out=const_sb[:, 1552:1808], in_=maskt)
            wo_sb = big.tile([128, 4, 1024], B16, tag="bigC")
            nc.sync.dma_start(out=wo_sb[:], in_=woT)

            # ---- persistent activations ----
            qt = pqt.tile([128, 4, 2048], B16)   # Q^T: u-dim on partitions
            kt = pkt.tile([128, 4, 2048], B16)   # K^T
            vt = pv.tile([128, 16, 520], B16)    # V: [n part, 8*(64+ones)]

            # 5 phases: phase p emits projections for slabs (2p, 2p+1)
            # INTERLEAVED with the attention of q-chunk p-1; the O-proj of
            # chunk p-2 rides along one phase later (its normalize is long
            # done by then, so it never stalls the PE).
            def proj_units(sp):
                units = []

                def mk_dma(ns):
                    def dma_u():
                        xs = pxs.tile([128, 8, 256], B16, tag="xs", name=f"xs{ns}")
                        nc.sync.dma_start(out=xs[:, 0:4], in_=xT[ns, :, 0:4])
                        nc.sync.dma_start(out=xs[:, 4:8], in_=xT[ns, :, 4:8])
                        xss[ns] = xs
                    return dma_u

                def mk_v(ns, nn):
                    def v_u():
                        ni = 2 * ns + nn
                        pmv = pprj.tile([128, 512], F32, tag="mm", name="pmv")
                        for kc in range(8):
                            nc.tensor.matmul(
                                pmv[:],
                                xss[ns][:, kc, 128 * nn:128 * (nn + 1)],
                                wv_sb[:, kc, :],
                                start=(kc == 0), stop=(kc == 7),
                            )
                        vslab = vt[:, ni, :].rearrange("p (h e) -> p h e", e=65)
                        nc.vector.tensor_copy(out=vslab[:, :, 64], in_=ones8_sb)
                        nc.vector.tensor_add(
                            vslab[:, :, 0:64],
                            pmv[:].rearrange("p (h e) -> p h e", e=64),
                            bvb_sb.rearrange("p (h e) -> p h e", e=64),
                        )
                    return v_u

                def mk_qk(ns, oc):
                    def qk_u():
                        pm = pprj.tile([128, 256], F32, tag="mm", name="pmqk")
                        for kc in range(8):
                            nc.tensor.matmul(
                                pm[:],
                                wqk_sb[:, kc, 128 * oc:128 * (oc + 1)],
                                xss[ns][:, kc, :],
                                start=(kc == 0), stop=(kc == 7),
                            )
                        dest = qt if oc < 4 else kt
                        nc.vector.tensor_scalar_add(
                            dest[:, oc % 4, 256 * ns:256 * (ns + 1)], pm[:],
                            bqk_sb[:, oc:oc + 1],
                        )
                    return qk_u

                # x slabs for THIS phase were prefetched last phase; here we
                # prefetch the next phase's two slabs.
                if sp == 0:
                    # V first: needs only xs+wv (the first DMAs to land);
                    # the QK units then overlap the wqk stream.
                    units.extend([mk_v(0, 0), mk_v(0, 1), mk_v(1, 0), mk_v(1, 1)])
                    units.append(mk_dma(2))
                    units.extend([mk_qk(0, oc) for oc in range(8)])
                    units.append(mk_dma(3))
                    units.extend([mk_qk(1, oc) for oc in range(8)])
                else:
                    if sp < 3:
                        units.append(mk_dma(2 * sp + 2))
                    units.extend([mk_qk(2 * sp, oc) for oc in range(8)])
                    if sp < 3:
                        units.append(mk_dma(2 * sp + 3))
                    units.extend([mk_qk(2 * sp + 1, oc) for oc in range(8)])
                    units.extend([mk_v(2 * sp, 0), mk_v(2 * sp, 1),
                                  mk_v(2 * sp + 1, 0), mk_v(2 * sp + 1, 1)])
                return units

            def attn_units(sp, atq, hps=(0, 1, 2, 3), piecewise_norm=(),
                           batched=False, nb_sink=None):
                q0 = 512 * sp
                J = 4 * (sp + 1)
                LAG = lag   # PV of step j is emitted inside step j+LAG's unit
                units = []
                # Sequential head-pair chains; each chain's PV runs LAG steps
                # behind its S_T/exp so the PE never waits on the exp.
                batch = {"rd4": None, "rows": []}

                def mk_norm_batch(batch=batch):
                    # One DVE reciprocal for 4 collected denominators (at
                    # partition bases 0/32/64/96), then broadcast + in-place
                    # multiply of the already-copied bf16 numerators in atq.
                    def nb_u():
                        rd4 = batch["rd4"]
                        rr4 = prd.tile([128, 512], F32, tag="rr4", name="rr4")
                        nc.vector.reciprocal(rr4[:], rd4[:])
                        for (hp, half, r, aun) in batch["rows"]:
                            po = 64 * half
                            # partition_broadcast only works from/to base 0
                            # on HW: stage the row at base 0 via ACT first.
                            st0 = prd.tile([1, 512], F32, tag="st", name="st")
                            nc.scalar.activation(out=st0[:], in_=rr4[r:r + 1, :],
                                                 func=Copy)
                            rb = precb.tile([128, 512], F32, tag="rb", name="rb")
                            nc.gpsimd.partition_broadcast(rb[0:64, :], st0[:])
                            nc.vector.tensor_mul(
                                atq[po:po + 64, hp, :],
                                aun[0:64, :], rb[0:64, :])
                        batch["rd4"] = None
                        batch["rows"] = []
                    return nb_u

                for hp in hps:
                    st = {"pvps": None, "pend": []}

                    def norm_piece(hp, st, lo, hi):
                        for half in range(2):
                            po = 64 * half
                            pvp = st["pvps"][half]
                            rd = prd.tile([1, 512], F32, tag="rd", name="rd")
                            nc.vector.reciprocal(rd[:, lo:hi],
                                                 pvp[64:65, lo:hi])
                            rb = precb.tile([128, 512], F32, tag="rb", name="rb")
                            nc.gpsimd.partition_broadcast(rb[0:64, lo:hi],
                                                          rd[:, lo:hi])
                            nc.vector.tensor_mul(
                                atq[po:po + 64, hp, lo:hi],
                                pvp[0:64, lo:hi], rb[0:64, lo:hi])

                    def norm_defer(hp, st, batch=batch):
                        # Chain end for a batched chunk: ACT-copies the bf16
                        # numerator into atq and the denominator into the
                        # shared rd4 collection tile; division happens in the
                        # next norm-batch unit (off the critical path).
                        if batch["rd4"] is None:
                            batch["rd4"] = prd4.tile([128, 512], F32,
                                                     tag="rd4", name="rd4")
                            nc.vector.memset(batch["rd4"][:], 1.0)
                        for half in range(2):
                            pvp = st["pvps"][half]
                            aun = paun.tile([64, 512], B16, tag="aun",
                                            name="aun")
                            nc.scalar.activation(
                                out=aun[:], in_=pvp[0:64, :], func=Copy)
                            r = 64 * (hp % 2) + 32 * half
                            nc.scalar.activation(
                                out=batch["rd4"][r:r + 1, :],
                                in_=pvp[64:65, :], func=Copy)
                            batch["rows"].append((hp, half, r, aun))

                    def emit_pv(hp, st, last, piecewise=False):
                        pj, ppt, pc0 = st["pend"].pop(0)
                        if pj == 0:
                            st["pvps"] = [
                                pspv.tile([65, 512], F32, tag="pv", name="pvpa"),
                                pspv.tile([65, 512], F32, tag="pv", name="pvpb"),
                            ]
                        for half in range(2):
                            h = 2 * hp + half
                            nc.tensor.matmul(
                                st["pvps"][half][:, pc0:512],
                                vt[:, pj, 65 * h:65 * h + 65],
                                ppt[:, half, pc0:512],
                                start=(pj == 0), stop=last,
                            )
                        # columns [128*toff, 128*toff+128) got their final PV
                        # contribution: normalize them right away so the
                        # O-projection never waits on a monolithic normalize.
                        if piecewise:
                            toff = pj - 4 * sp
                            if toff >= 0:
                                norm_piece(hp, st, 128 * toff, 128 * toff + 128)

                    def mk_step(hp, j, st=st):
                        def step_u():
                            toff = j - 4 * sp
                            c0 = 128 * toff if toff > 0 else 0
                            sm = psmm.tile([128, 2, 512], F32, tag="sm", name="sm")
                            for half in range(2):  # head 2hp+half in PE band
                                po = 64 * half
                                nc.tensor.matmul(
                                    sm[:, half, c0:512],
                                    kt[po:po + 64, hp, 128 * j:128 * (j + 1)],
                                    qt[po:po + 64, hp, q0 + c0:q0 + 512],
                                    start=True, stop=True,
                                )
                            if toff >= 0:  # diagonal: triangle add (both)
                                nc.vector.tensor_add(
                                    sm[:, :, c0:c0 + 128], sm[:, :, c0:c0 + 128],
                                    tri2_sb)
                            pt = pprob.tile([128, 2, 512], B16, tag="pt", name="pt")
                            nc.scalar.activation(
                                out=pt[:, :, c0:512], in_=sm[:, :, c0:512],
                                func=Exp, scale=0.125)
                            st["pend"].append((j, pt, c0))
                            if len(st["pend"]) > LAG:
                                emit_pv(hp, st, last=False,
                                        piecewise=hp in piecewise_norm)
                        return step_u

                    def mk_flush(hp, st=st):
                        def flush_u():
                            pw = hp in piecewise_norm
                            while st["pend"]:
                                emit_pv(hp, st, last=not st["pend"][1:],
                                        piecewise=pw)
                            if pw:
                                return
                            if batched:
                                norm_defer(hp, st)
                            else:  # normalize both heads in one go
                                norm_piece(hp, st, 0, 512)
                        return flush_u

                    for j in range(J):
                        units.append(mk_step(hp, j))
                    units.append(mk_flush(hp))
                    if batched and hp % 2 == 1:
                        nb = mk_norm_batch()
                        if hp == 3 and nb_sink is not None:
                            nb_sink.append(nb)   # run early next phase
                        else:
                            units.append(nb)
                return units

            def o_units(sp, atq):
                units = []
                for k in range(4):
                    for oh in range(2):
                        def o_u(k=k, oh=oh):
                            ni = 4 * sp + k
                            pm = pprj.tile([128, 512], F32, tag="mm", name="pmo")
                            for uc in range(4):
                                nc.tensor.matmul(
                                    pm[:],
                                    atq[:, uc, 128 * k:128 * (k + 1)],
                                    wo_sb[:, uc, 512 * oh:512 * (oh + 1)],
                                    start=(uc == 0), stop=(uc == 3),
                                )
                            ob = pout.tile([128, 512], B16, tag="ob", name="ob")
                            nc.vector.tensor_add(
                                ob[:], pm[:], bob_sb[:, 512 * oh:512 * (oh + 1)])
                            nc.scalar.dma_start(
                                out=outr[:, ni, 512 * oh:512 * (oh + 1)], in_=ob[:])
                        units.append(o_u)
                return units

            def o_units_split(sp, atq):
                """O-proj split in two half-accumulations: the uc 0/1 part
                can run while head-pairs 2/3 are still in attention."""
                obas = {}
                ua, ub = [], []
                for k in range(4):
                    for oh in range(2):
                        def oa_u(k=k, oh=oh):
                            pm = pprj.tile([128, 512], F32, tag="mm", name="pmoa")
                            for uc in range(2):
                                nc.tensor.matmul(
                                    pm[:],
                                    atq[:, uc, 128 * k:128 * (k + 1)],
                                    wo_sb[:, uc, 512 * oh:512 * (oh + 1)],
                                    start=(uc == 0), stop=(uc == 1),
                                )
                            oba = poba.tile([128, 512], B16, tag="oba", name="oba")
                            nc.vector.tensor_add(
                                oba[:], pm[:], bob_sb[:, 512 * oh:512 * (oh + 1)])
                            obas[(k, oh)] = oba

                        def ob_u(k=k, oh=oh):
                            ni = 4 * sp + k
                            pm = pprj.tile([128, 512], F32, tag="mm", name="pmob")
                            for uc in range(2, 4):
                                nc.tensor.matmul(
                                    pm[:],
                                    atq[:, uc, 128 * k:128 * (k + 1)],
                                    wo_sb[:, uc, 512 * oh:512 * (oh + 1)],
                                    start=(uc == 2), stop=(uc == 3),
                                )
                            ob = pout.tile([128, 512], B16, tag="ob", name="ob")
                            nc.vector.tensor_add(ob[:], pm[:], obas[(k, oh)][:])
                            nc.scalar.dma_start(
                                out=outr[:, ni, 512 * oh:512 * (oh + 1)], in_=ob[:])
                        ua.append(oa_u)
                        ub.append(ob_u)
                return ua, ub

            def run_interleaved(cur, prev):
                # proportional round-robin interleave of cur and prev
                na, nb = len(cur), len(prev)
                ia = ib = 0
                while ia < na or ib < nb:
                    if ib * max(na, 1) * ratio[1] <= ia * max(nb, 1) * ratio[0] and ib < nb or ia >= na:
                        prev[ib](); ib += 1
                    else:
                        cur[ia](); ia += 1

            xss = {0: xs0, 1: xs1}
            atqs = {}
            for sp in range(1, 4):
                atqs[sp - 1] = None
            nb_pend = []
            for sp in range(4):
                cur = list(nb_pend)
                nb_pend = []
                cur += proj_units(sp)
                if sp >= 2:
                    cur = cur + o_units(sp - 2, atqs[sp - 2])
                prev = []
                if sp >= 1:
                    atqs[sp - 1] = patq.tile([128, 4, 512], B16, tag="atq",
                                             name=f"atq{sp - 1}")
                    prev = attn_units(sp - 1, atqs[sp - 1], batched=True,
                                      nb_sink=nb_pend)
                run_interleaved(cur, prev)
            # phase 4: attention chunk 3; O(2) rides the first two chains,
            # O(3)'s uc0/1 half rides the last two, its uc2/3 half drains.
            atqs[3] = patq.tile([128, 4, 512], B16, tag="atq", name="atq3")
            o3a, o3b = o_units_split(3, atqs[3])
            run_interleaved(nb_pend + o_units(2, atqs[2]),
                            attn_units(3, atqs[3], hps=(0, 1), batched=True))
            run_interleaved(o3a,
                            attn_units(3, atqs[3], hps=(2, 3),
                                       piecewise_norm=(2, 3)))
            for u in o3b:
                u()

    nc.compile()
    return nc


_NC_CACHE = {}


def _get_nc(repeat=1, **kw):
    key = (repeat, tuple(sorted(kw.items())))
    if key not in _NC_CACHE:
        _NC_CACHE[key] = _build(repeat, **kw)
    return _NC_CACHE[key]


def _host_inputs(x, Wq, bq, Wk, bk, Wv, bv, Wo, bo):
    """Build the 8 per-core input maps."""
    import ml_dtypes
    f32 = np.float32
    B16 = ml_dtypes.bfloat16

    def rnd(a):
        return np.ascontiguousarray(a, dtype=f32).astype(B16)

    r = np.arange(128)[:, None]
    c = np.arange(128)[None, :]
    mask1 = np.where(r <= c, f32(0.0), f32(-1e4)).astype(f32)
    mask = np.concatenate([mask1, mask1], axis=1)

    in_maps = []
    for core in range(8):
        bi, hg = core // 2, core % 2
        hsl = slice(512 * hg, 512 * (hg + 1))
        # xT swizzled: [ns, p, kc, col] = x[bi].T[kc*128+p, 256*ns+col]
        xTl = rnd(np.ascontiguousarray(
            x[bi].T.reshape(8, 128, 8, 256).transpose(2, 1, 0, 3)))
        wqkTl = rnd(np.ascontiguousarray(
            np.concatenate([Wq[hsl].T, Wk[hsl].T], axis=1).reshape(8, 128, 1024)
            .transpose(1, 0, 2)))
        wvTl = rnd(np.ascontiguousarray(
            Wv[hsl].T.reshape(8, 128, 512).transpose(1, 0, 2)))
        woTl = rnd(np.ascontiguousarray(
            Wo[:, hsl].T.reshape(4, 128, 1024).transpose(1, 0, 2)))
        bq_l, bk_l = bq[hsl], bk[hsl]
        bqk_t = np.stack(
            [bq_l[128 * i:128 * (i + 1)] for i in range(4)]
            + [bk_l[128 * i:128 * (i + 1)] for i in range(4)]
            + [np.ones(128, dtype=f32)] * 8, axis=1
        ).astype(f32)
        bvb_t = np.broadcast_to(bv[hsl].astype(f32), (128, 512)).copy()
        if hg == 0:
            bob_t = np.broadcast_to(bo.astype(f32), (128, 1024)).copy()
        else:
            bob_t = np.zeros((128, 1024), dtype=f32)
        in_maps.append({
            "xT": xTl, "wqkT": wqkTl, "wvT": wvTl, "woT": woTl,
            "bqk": bqk_t, "bvb": bvb_t, "bob": bob_t, "maskt": mask,
        })
    return in_maps


def kernel(x, Wq, bq, Wk, bk, Wv, bv, Wo, bo):
    from concourse.bass_utils import run_bass_kernel_spmd

    x = np.asarray(x); Wq = np.asarray(Wq); bq = np.asarray(bq)
    Wk = np.asarray(Wk); bk = np.asarray(bk); Wv = np.asarray(Wv)
    bv = np.asarray(bv); Wo = np.asarray(Wo); bo = np.asarray(bo)

    nc = _get_nc()
    in_maps = _host_inputs(x, Wq, bq, Wk, bk, Wv, bv, Wo, bo)
    r = run_bass_kernel_spmd(nc, in_maps, list(range(8)))

    out = np.empty((4, 2048, 1024), dtype=np.float32)
    for bi in range(4):
        out[bi] = (r.results[2 * bi]["out"].astype(np.float32)
                   + r.results[2 * bi + 1]["out"].astype(np.float32))
    return out


# revision 47
# speedup vs baseline: 1.3868x; 1.1949x over previous
"""Causal self-attention (b=4, s=2048, d=1024, h=16, hd=64) on 8 trn2 cores.

Sharding: (batch, head-group) — core c handles batch c//2 and heads
[8*(c%2), 8*(c%2)+8) (Megatron column-parallel QKV + row-parallel O).
Each core returns a partial (2048, 1024) bf16 output for its batch; the
host upcasts and sums the two partials per batch (the row-parallel
reduce of the Megatron pattern, done as part of unsharding).

All matmul operands are bf16 (full PE rate like fp32r, half the
DMA/SBUF traffic, and no N>=256 full-rate constraint); accumulation is
fp32 in PSUM, biases fp32.

Per-core device program (layouts chosen so NO on-chip transposes are
needed):
    xT (1024,2048) = x[b].T feeds both Q^T/K^T (as moving operand) and
    V (as stationary operand).  Q^T/K^T stored [o=512 part-dims, n];
    V stored [n part, o free] with a ones column per head so the softmax
    denominator falls out of the PV matmul (M=65).  probs kept
    TRANSPOSED [kv, q]: softmax needs no max-subtraction (scores bounded
    ~|3|), the causal mask is additive (-1e4 pre-exp, exp underflows to
    0), and attn^T [u, n] is directly the stationary operand of the
    O-projection.  Causality: fully-masked kv-chunks are skipped
    entirely; on diagonal chunks only the live column range is computed.

    Schedule: 5 phases; phase p emits the projections of x-slabs
    (2p, 2p+1) interleaved with the attention of q-chunk p-1; the
    O-projection of chunk p-2 rides along one phase later so it never
    waits on the softmax normalize.  Attention runs as sequential
    head-pair chains whose PV matmuls lag the S_T/exp stream by `lag`
    steps, so the in-order PE never waits on the ACT exp.  Scores for a
    head pair land in ONE merged [128,2,512] PSUM tile, so mask-add and
    exp are single instructions over both heads.  PSUM: 2 merged score
    tiles (4 banks) + 2 projection accumulators (2 banks) + 2 PV
    accumulator banks.

    Softmax normalize: nc.vector.reciprocal is ~2.8us on HW (multi-pass)
    and blocks the in-order DVE queue, so denominators are collected
    across two chains into one [128,512] tile (rows 0/32/64/96 — the
    only partition bases single-row ACT/DVE ops support) via cheap ACT
    copies, reciprocal'd in ONE DVE op, then broadcast+multiplied into
    the (ACT-copied) bf16 numerators off the critical path.  The last
    chunk's final chain instead normalizes piecewise per 128-column
    group as its diagonal PVs complete, so the O-projection tail starts
    immediately.  (gpsimd partition_broadcast only works from/to
    partition base 0 on HW; ACT stages rows there first.)

    The final chunk's O-projection is split into uc{0,1} and uc{2,3}
    half-accumulations so its first half overlaps the last two
    attention chains.

    Measured (NTFF, single dispatch, core 0): ~331 us at throttle-util
    0.85 (~282 us unthrottled-equivalent) vs 577 us (at util 0.60;
    ~349 us equivalent) for the fp32r predecessor.  Cost-model
    prediction (no throttle): ~270 us.  rel err vs fp32 reference:
    4.1e-3 (threshold 2e-2).
"""
from contextlib import ExitStack

import numpy as np

MM_MODE = "bf16"  # kept for test.py compat; only bf16 path exists


def _build(repeat=1, ratio=(2, 1), lag=3):
    import concourse.tile as tile
    from concourse import bacc, mybir

    dt = mybir.dt
    F32 = dt.float32
    B16 = dt.bfloat16
    Exp = mybir.ActivationFunctionType.Exp
    Copy = mybir.ActivationFunctionType.Copy
    Identity = mybir.ActivationFunctionType.Identity

    nc = bacc.Bacc("TRN2", target_bir_lowering=False, debug=False, num_devices=8)

    xT = nc.dram_tensor("xT", [8, 128, 8, 256], B16, kind="ExternalInput").ap()
    wqkT = nc.dram_tensor("wqkT", [128, 8, 1024], B16, kind="ExternalInput").ap()
    wvT = nc.dram_tensor("wvT", [128, 8, 512], B16, kind="ExternalInput").ap()
    woT = nc.dram_tensor("woT", [128, 4, 1024], B16, kind="ExternalInput").ap()
    bqk = nc.dram_tensor("bqk", [128, 16], F32, kind="ExternalInput").ap()
    bvb = nc.dram_tensor("bvb", [128, 512], F32, kind="ExternalInput").ap()
    bob = nc.dram_tensor("bob", [128, 1024], F32, kind="ExternalInput").ap()
    maskt = nc.dram_tensor("maskt", [128, 256], F32, kind="ExternalInput").ap()
    out = nc.dram_tensor("out", [2048, 1024], B16, kind="ExternalOutput").ap()

    outr = out.rearrange("(nc p) o -> p nc o", p=128)    # [128, 16, 1024]

    with tile.TileContext(nc) as tc, ExitStack() as ctx:
        big = ctx.enter_context(tc.tile_pool(name="big", bufs=1))
        pqt = ctx.enter_context(tc.tile_pool(name="pqt", bufs=1))
        pkt = ctx.enter_context(tc.tile_pool(name="pkt", bufs=1))
        pv = ctx.enter_context(tc.tile_pool(name="pv", bufs=1))
        pxs = ctx.enter_context(tc.tile_pool(name="pxs", bufs=3))
        pprob = ctx.enter_context(tc.tile_pool(name="pprob", bufs=4))
        precb = ctx.enter_context(tc.tile_pool(name="precb", bufs=1))
        prd = ctx.enter_context(tc.tile_pool(name="prd", bufs=1))
        prd4 = ctx.enter_context(tc.tile_pool(name="prd4", bufs=3))
        pone = ctx.enter_context(tc.tile_pool(name="pone", bufs=1))
        pout = ctx.enter_context(tc.tile_pool(name="pout", bufs=2))
        poba = ctx.enter_context(tc.tile_pool(name="poba", bufs=8))
        paun = ctx.enter_context(tc.tile_pool(name="paun", bufs=8))
        patq = ctx.enter_context(tc.tile_pool(name="patq", bufs=2))
        psmm = ctx.enter_context(tc.tile_pool(name="psmm", bufs=2, space="PSUM"))
        pprj = ctx.enter_context(tc.tile_pool(name="pprj", bufs=2, space="PSUM"))
        pspv = ctx.enter_context(tc.tile_pool(name="pspv", bufs=2, space="PSUM"))

        # ---- constants (one merged tile: bqk | ones8 | bvb | bob | mask2) ----
        const_sb = pone.tile([128, 1808], F32, tag="const")
        bqk_sb = const_sb[:, 0:8]
        ones8_sb = const_sb[:, 8:16]
        bvb_sb = const_sb[:, 16:528]
        bob_sb = const_sb[:, 528:1552]
        tri2_sb = const_sb[:, 1552:1808].rearrange("p (two e) -> p two e", e=128)

        for rep in range(repeat):
            # prefetch the first two x slabs so projections start ASAP
            xs0 = pxs.tile([128, 8, 256], B16, tag="xs")
            nc.sync.dma_start(out=xs0[:, 0:4], in_=xT[0, :, 0:4])
            # ---- weights ----
            wv_sb = big.tile([128, 8, 512], B16, tag="bigB")
            nc.sync.dma_start(out=wv_sb[:, 0:4], in_=wvT[:, 0:4])
            nc.sync.dma_start(out=xs0[:, 4:8], in_=xT[0, :, 4:8])
            nc.sync.dma_start(out=wv_sb[:, 4:8], in_=wvT[:, 4:8])
            if rep == 0:
                nc.sync.dma_start(out=const_sb[:, 0:16], in_=bqk)
                nc.sync.dma_start(out=bvb_sb, in_=bvb)
            xs1 = pxs.tile([128, 8, 256], B16, tag="xs", name="xs1")
            nc.sync.dma_start(out=xs1[:, 0:4], in_=xT[1, :, 0:4])
            nc.sync.dma_start(out=xs1[:, 4:8], in_=xT[1, :, 4:8])
            wqk_sb = big.tile([128, 8, 1024], B16, tag="bigA")
            for kc in range(8):
                nc.sync.dma_start(out=wqk_sb[:, kc], in_=wqkT[:, kc])
            if rep == 0:
                nc.sync.dma_start(out=bob_sb, in_=bob)
                nc.sync.dma_start(out=const_sb[:, 1552:1808], in_=maskt)
            wo_sb = big.tile([128, 4, 1024], B16, tag="bigC")
            nc.sync.dma_start(out=wo_sb[:], in_=woT)

            # ---- persistent activations ----
            qt = pqt.tile([128, 4, 2048], B16)   # Q^T: u-dim on partitions
            kt = pkt.tile([128, 4, 2048], B16)   # K^T
            vt = pv.tile([128, 16, 520], B16)    # V: [n part, 8*(64+ones)]

            # 5 phases: phase p emits projections for slabs (2p, 2p+1)
            # INTERLEAVED with the attention of q-chunk p-1; the O-proj of
            # chunk p-2 rides along one phase later (its normalize is long
            # done by then, so it never stalls the PE).
            def proj_units(sp):
                units = []

                def mk_dma(ns):
                    def dma_u():
                        xs = pxs.tile([128, 8, 256], B16, tag="xs", name=f"xs{ns}")
                        nc.sync.dma_start(out=xs[:, 0:4], in_=xT[ns, :, 0:4])
                        nc.sync.dma_start(out=xs[:, 4:8], in_=xT[ns, :, 4:8])
                        xss[ns] = xs
                    return dma_u

                def mk_v(ns, nn):
                    def v_u():
                        ni = 2 * ns + nn
                        pmv = pprj.tile([128, 512], F32, tag="mm", name="pmv")
                        for kc in range(8):
                            nc.tensor.matmul(
                                pmv[:],
                                xss[ns][:, kc, 128 * nn:128 * (nn + 1)],
                                wv_sb[:, kc, :],
                                start=(kc == 0), stop=(kc == 7),
                            )
                        vslab = vt[:, ni, :].rearrange("p (h e) -> p h e", e=65)
                        nc.vector.tensor_copy(out=vslab[:, :, 64], in_=ones8_sb)
                        nc.vector.tensor_add(
                            vslab[:, :, 0:64],
                            pmv[:].rearrange("p (h e) -> p h e", e=64),
                            bvb_sb.rearrange("p (h e) -> p h e", e=64),
                        )
                    return v_u

                def mk_qk(ns, oc):
                    def qk_u():
                        pm = pprj.tile([128, 256], F32, tag="mm", name="pmqk")
                        for kc in range(8):
                            nc.tensor.matmul(
                                pm[:],
                                wqk_sb[:, kc, 128 * oc:128 * (oc + 1)],
                                xss[ns][:, kc, :],
                                start=(kc == 0), stop=(kc == 7),
                            )
                        dest = qt if oc < 4 else kt
                        nc.vector.tensor_scalar_add(
                            dest[:, oc % 4, 256 * ns:256 * (ns + 1)], pm[:],
                            bqk_sb[:, oc:oc + 1],
                        )
                    return qk_u

                # x slabs for THIS phase were prefetched last phase; here we
                # prefetch the next phase's two slabs.
                if sp == 0:
                    # V first: needs only xs+wv (the first DMAs to land);
                    # the QK units then overlap the wqk stream.
                    units.extend([mk_v(0, 0), mk_v(0, 1), mk_v(1, 0), mk_v(1, 1)])
                    units.append(mk_dma(2))
                    units.extend([mk_qk(0, oc) for oc in range(8)])
                    units.append(mk_dma(3))
                    units.extend([mk_qk(1, oc) for oc in range(8)])
                else:
                    if sp < 3:
                        units.append(mk_dma(2 * sp + 2))
                    units.extend([mk_qk(2 * sp, oc) for oc in range(8)])
                    if sp < 3:
                        units.append(mk_dma(2 * sp + 3))
                    units.extend([mk_qk(2 * sp + 1, oc) for oc in range(8)])
                    units.extend([mk_v(2 * sp, 0), mk_v(2 * sp, 1),
                                  mk_v(2 * sp + 1, 0), mk_v(2 * sp + 1, 1)])
                return units

            def attn_units(sp, atq, hps=(0, 1, 2, 3), piecewise_norm=(),
                           batched=False, nb_sink=None):
                q0 = 512 * sp
                J = 4 * (sp + 1)
                LAG = lag   # PV of step j is emitted inside step j+LAG's unit
                units = []
                # Sequential head-pair chains; each chain's PV runs LAG steps
                # behind its S_T/exp so the PE never waits on the exp.
                batches = {0: {"rd4": None, "rows": []},
                           1: {"rd4": None, "rows": []}}

                def mk_norm_batch(batch):
                    # One DVE reciprocal for 4 collected denominators (at
                    # partition bases 0/32/64/96), then broadcast + multiply
                    # of the ACT-copied bf16 numerators into atq.  Each
                    # chain-pair has its own batch dict, so this unit can
                    # safely execute a phase after it was emitted.
                    def nb_u():
                        rd4 = batch["rd4"]
                        rows = batch["rows"]
                        rr4 = prd.tile([128, 512], F32, tag="rr4", name="rr4")
                        nc.vector.reciprocal(rr4[:], rd4[:])
                        for (hp, half, r, aun) in rows:
                            po = 64 * half
                            # partition_broadcast only works from/to base 0
                            # on HW: stage the row at base 0 via ACT first.
                            st0 = prd.tile([1, 512], F32, tag="st", name="st")
                            nc.scalar.activation(out=st0[:], in_=rr4[r:r + 1, :],
                                                 func=Copy)
                            rb = precb.tile([128, 512], F32, tag="rb", name="rb")
                            nc.gpsimd.partition_broadcast(rb[0:64, :], st0[:])
                            nc.vector.tensor_mul(
                                atq[po:po + 64, hp, :],
                                aun[0:64, :], rb[0:64, :])
                    return nb_u

                for hp in hps:
                    st = {"pvps": None, "pend": []}

                    def norm_piece(hp, st, lo, hi):
                        for half in range(2):
                            po = 64 * half
                            pvp = st["pvps"][half]
                            rd = prd.tile([1, 512], F32, tag="rd", name="rd")
                            nc.vector.reciprocal(rd[:, lo:hi],
                                                 pvp[64:65, lo:hi])
                            rb = precb.tile([128, 512], F32, tag="rb", name="rb")
                            nc.gpsimd.partition_broadcast(rb[0:64, lo:hi],
                                                          rd[:, lo:hi])
                            nc.vector.tensor_mul(
                                atq[po:po + 64, hp, lo:hi],
                                pvp[0:64, lo:hi], rb[0:64, lo:hi])

                    def norm_defer(hp, st):
                        batch = batches[hp // 2]
                        # Chain end for a batched chunk: ACT-copies the bf16
                        # numerator into atq and the denominator into the
                        # shared rd4 collection tile; division happens in the
                        # next norm-batch unit (off the critical path).
                        if batch["rd4"] is None:
                            batch["rd4"] = prd4.tile([128, 512], F32,
                                                     tag="rd4", name="rd4")
                            nc.vector.memset(batch["rd4"][:], 1.0)
                        for half in range(2):
                            pvp = st["pvps"][half]
                            aun = paun.tile([64, 512], B16, tag="aun",
                                            name="aun")
                            nc.scalar.activation(
                                out=aun[:], in_=pvp[0:64, :], func=Copy)
                            r = 64 * (hp % 2) + 32 * half
                            nc.scalar.activation(
                                out=batch["rd4"][r:r + 1, :],
                                in_=pvp[64:65, :], func=Copy)
                            batch["rows"].append((hp, half, r, aun))

                    def emit_pv(hp, st, last, piecewise=False):
                        pj, ppt, pc0 = st["pend"].pop(0)
                        if pj == 0:
                            st["pvps"] = [
                                pspv.tile([65, 512], F32, tag="pv", name="pvpa"),
                                pspv.tile([65, 512], F32, tag="pv", name="pvpb"),
                            ]
                        for half in range(2):
                            h = 2 * hp + half
                            nc.tensor.matmul(
                                st["pvps"][half][:, pc0:512],
                                vt[:, pj, 65 * h:65 * h + 65],
                                ppt[:, half, pc0:512],
                                start=(pj == 0), stop=last,
                            )
                        # columns [128*toff, 128*toff+128) got their final PV
                        # contribution: normalize them right away so the
                        # O-projection never waits on a monolithic normalize.
                        if piecewise:
                            toff = pj - 4 * sp
                            if toff >= 0:
                                norm_piece(hp, st, 128 * toff, 128 * toff + 128)

                    def mk_step(hp, j, st=st):
                        def step_u():
                            toff = j - 4 * sp
                            c0 = 128 * toff if toff > 0 else 0
                            sm = psmm.tile([128, 2, 512], F32, tag="sm", name="sm")
                            for half in range(2):  # head 2hp+half in PE band
                                po = 64 * half
                                nc.tensor.matmul(
                                    sm[:, half, c0:512],
                                    kt[po:po + 64, hp, 128 * j:128 * (j + 1)],
                                    qt[po:po + 64, hp, q0 + c0:q0 + 512],
                                    start=True, stop=True,
                                )
                            if toff >= 0:  # diagonal: triangle add (both)
                                nc.vector.tensor_add(
                                    sm[:, :, c0:c0 + 128], sm[:, :, c0:c0 + 128],
                                    tri2_sb)
                            pt = pprob.tile([128, 2, 512], B16, tag="pt", name="pt")
                            nc.scalar.activation(
                                out=pt[:, :, c0:512], in_=sm[:, :, c0:512],
                                func=Exp, scale=0.125)
                            st["pend"].append((j, pt, c0))
                            if len(st["pend"]) > LAG:
                                emit_pv(hp, st, last=False,
                                        piecewise=hp in piecewise_norm)
                        return step_u

                    def mk_flush(hp, st=st):
                        def flush_u():
                            pw = hp in piecewise_norm
                            while st["pend"]:
                                emit_pv(hp, st, last=not st["pend"][1:],
                                        piecewise=pw)
                            if pw:
                                return
                            if batched:
                                norm_defer(hp, st)
                            else:  # normalize both heads in one go
                                norm_piece(hp, st, 0, 512)
                        return flush_u

                    for j in range(J):
                        units.append(mk_step(hp, j))
                    units.append(mk_flush(hp))
                    if batched and hp % 2 == 1:
                        nb = mk_norm_batch(batches[hp // 2])
                        if nb_sink is not None:
                            nb_sink.append(nb)   # run early next phase
                        else:
                            units.append(nb)
                return units

            def o_units(sp, atq):
                units = []
                for k in range(4):
                    for oh in range(2):
                        def o_u(k=k, oh=oh):
                            ni = 4 * sp + k
                            pm = pprj.tile([128, 512], F32, tag="mm", name="pmo")
                            for uc in range(4):
                                nc.tensor.matmul(
                                    pm[:],
                                    atq[:, uc, 128 * k:128 * (k + 1)],
                                    wo_sb[:, uc, 512 * oh:512 * (oh + 1)],
                                    start=(uc == 0), stop=(uc == 3),
                                )
                            ob = pout.tile([128, 512], B16, tag="ob", name="ob")
                            nc.vector.tensor_add(
                                ob[:], pm[:], bob_sb[:, 512 * oh:512 * (oh + 1)])
                            nc.scalar.dma_start(
                                out=outr[:, ni, 512 * oh:512 * (oh + 1)], in_=ob[:])
                        units.append(o_u)
                return units

            def o_units_split(sp, atq):
                """O-proj split in two half-accumulations: the uc 0/1 part
                can run while head-pairs 2/3 are still in attention."""
                obas = {}
                ua, ub = [], []
                for k in range(4):
                    for oh in range(2):
                        def oa_u(k=k, oh=oh):
                            pm = pprj.tile([128, 512], F32, tag="mm", name="pmoa")
                            for uc in range(2):
                                nc.tensor.matmul(
                                    pm[:],
                                    atq[:, uc, 128 * k:128 * (k + 1)],
                                    wo_sb[:, uc, 512 * oh:512 * (oh + 1)],
                                    start=(uc == 0), stop=(uc == 1),
                                )
                            oba = poba.tile([128, 512], B16, tag="oba", name="oba")
                            nc.vector.tensor_add(
                                oba[:], pm[:], bob_sb[:, 512 * oh:512 * (oh + 1)])
                            obas[(k, oh)] = oba

                        def ob_u(k=k, oh=oh):
                            ni = 4 * sp + k
                            pm = pprj.tile([128, 512], F32, tag="mm", name="pmob")
                            for uc in range(2, 4):
                                nc.tensor.matmul(
                                    pm[:],
                                    atq[:, uc, 128 * k:128 * (k + 1)],
                                    wo_sb[:, uc, 512 * oh:512 * (oh + 1)],
                                    start=(uc == 2), stop=(uc == 3),
                                )
                            ob = pout.tile([128, 512], B16, tag="ob", name="ob")
                            nc.vector.tensor_add(ob[:], pm[:], obas[(k, oh)][:])
                            nc.scalar.dma_start(
                                out=outr[:, ni, 512 * oh:512 * (oh + 1)], in_=ob[:])
                        ua.append(oa_u)
                        ub.append(ob_u)
                return ua, ub

            def run_interleaved(cur, prev):
                # proportional round-robin interleave of cur and prev
                na, nb = len(cur), len(prev)
                ia = ib = 0
                while ia < na or ib < nb:
                    if ib * max(na, 1) * ratio[1] <= ia * max(nb, 1) * ratio[0] and ib < nb or ia >= na:
                        prev[ib](); ib += 1
                    else:
                        cur[ia](); ia += 1

            xss = {0: xs0, 1: xs1}
            atqs = {}
            for sp in range(1, 4):
                atqs[sp - 1] = None
            nb_pend = []
            for sp in range(4):
                cur = list(nb_pend)
                nb_pend = []
                cur += proj_units(sp)
                if sp >= 2:
                    cur = cur + o_units(sp - 2, atqs[sp - 2])
                prev = []
                if sp >= 1:
                    atqs[sp - 1] = patq.tile([128, 4, 512], B16, tag="atq",
                                             name=f"atq{sp - 1}")
                    prev = attn_units(sp - 1, atqs[sp - 1], batched=True,
                                      nb_sink=nb_pend)
                run_interleaved(cur, prev)
            # phase 4: attention chunk 3; O(2) rides the first two chains,
            # O(3)'s uc0/1 half rides the last two, its uc2/3 half drains.
            atqs[3] = patq.tile([128, 4, 512], B16, tag="atq", name="atq3")
            o3a, o3b = o_units_split(3, atqs[3])
            run_interleaved(nb_pend + o_units(2, atqs[2]),
                            attn_units(3, atqs[3], hps=(0, 1), batched=True))
            run_interleaved(o3a,
                            attn_units(3, atqs[3], hps=(2, 3),
                                       piecewise_norm=(2, 3)))
            for u in o3b:
                u()

    nc.compile()
    return nc


_NC_CACHE = {}


def _get_nc(repeat=1, **kw):
    key = (repeat, tuple(sorted(kw.items())))
    if key not in _NC_CACHE:
        _NC_CACHE[key] = _build(repeat, **kw)
    return _NC_CACHE[key]


def _host_inputs(x, Wq, bq, Wk, bk, Wv, bv, Wo, bo):
    """Build the 8 per-core input maps."""
    import ml_dtypes
    f32 = np.float32
    B16 = ml_dtypes.bfloat16

    def rnd(a):
        return np.ascontiguousarray(a, dtype=f32).astype(B16)

    r = np.arange(128)[:, None]
    c = np.arange(128)[None, :]
    mask1 = np.where(r <= c, f32(0.0), f32(-1e4)).astype(f32)
    mask = np.concatenate([mask1, mask1], axis=1)

    in_maps = []
    for core in range(8):
        bi, hg = core // 2, core % 2
        hsl = slice(512 * hg, 512 * (hg + 1))
        # xT swizzled: [ns, p, kc, col] = x[bi].T[kc*128+p, 256*ns+col]
        xTl = rnd(np.ascontiguousarray(
            x[bi].T.reshape(8, 128, 8, 256).transpose(2, 1, 0, 3)))
        wqkTl = rnd(np.ascontiguousarray(
            np.concatenate([Wq[hsl].T, Wk[hsl].T], axis=1).reshape(8, 128, 1024)
            .transpose(1, 0, 2)))
        wvTl = rnd(np.ascontiguousarray(
            Wv[hsl].T.reshape(8, 128, 512).transpose(1, 0, 2)))
        woTl = rnd(np.ascontiguousarray(
            Wo[:, hsl].T.reshape(4, 128, 1024).transpose(1, 0, 2)))
        bq_l, bk_l = bq[hsl], bk[hsl]
        bqk_t = np.stack(
            [bq_l[128 * i:128 * (i + 1)] for i in range(4)]
            + [bk_l[128 * i:128 * (i + 1)] for i in range(4)]
            + [np.ones(128, dtype=f32)] * 8, axis=1
        ).astype(f32)
        bvb_t = np.broadcast_to(bv[hsl].astype(f32), (128, 512)).copy()
        if hg == 0:
            bob_t = np.broadcast_to(bo.astype(f32), (128, 1024)).copy()
        else:
            bob_t = np.zeros((128, 1024), dtype=f32)
        in_maps.append({
            "xT": xTl, "wqkT": wqkTl, "wvT": wvTl, "woT": woTl,
            "bqk": bqk_t, "bvb": bvb_t, "bob": bob_t, "maskt": mask,
        })
    return in_maps


def kernel(x, Wq, bq, Wk, bk, Wv, bv, Wo, bo):
    from concourse.bass_utils import run_bass_kernel_spmd

    x = np.asarray(x); Wq = np.asarray(Wq); bq = np.asarray(bq)
    Wk = np.asarray(Wk); bk = np.asarray(bk); Wv = np.asarray(Wv)
    bv = np.asarray(bv); Wo = np.asarray(Wo); bo = np.asarray(bo)

    nc = _get_nc()
    in_maps = _host_inputs(x, Wq, bq, Wk, bk, Wv, bv, Wo, bo)
    r = run_bass_kernel_spmd(nc, in_maps, list(range(8)))

    out = np.empty((4, 2048, 1024), dtype=np.float32)
    for bi in range(4):
        out[bi] = (r.results[2 * bi]["out"].astype(np.float32)
                   + r.results[2 * bi + 1]["out"].astype(np.float32))
    return out
